# revision 1
# baseline (speedup 1.0000x reference)
"""ConvAttention kernel for 8x TRN2 NeuronCores.

Model (per batch item b):
    q/k/v = grouped_conv1d(x_b, w, b, groups=8)        # [E, T] -> [E, T]
    S     = (q^T k) / sqrt(E)                          # [T, T]
    P     = softmax(S, axis=-1)
    y     = (P @ v^T) @ w_fc^T + b_fc                  # [T, E]

Sharding: pure data-parallel over batch B=8 -> 8 cores, weights replicated.

Per-core algorithm (no transposes, scores never leave the chip):
  * conv projections as block-diagonal [128,128] matmuls per tap, output in
    "ET" layout (channels on partitions) -- exactly what matmul wants for the
    scores contraction over E.
  * fc is pushed in front of attention by associativity:
        y = P_norm @ (v_c @ w_fc^T + 1*beff)   with beff = w_fc@bv + b_fc
    (v's conv bias bv commutes through the softmax-normalized P).
  * scores are computed TRANSPOSED (S^T tiles, lhsT=k-tile, rhs=q-chunk) so
    that after exp the tiles are directly the stationary operand of attn@v.
  * softmax without max-subtraction (scores ~ N(0,1), exp is safe in fp32);
    row sums come for free from a ones-column appended to vw -> normalization
    is a per-partition reciprocal+scale on the final [128, 256] tiles.
  * all matmuls in fp32r (full PE speed at N>=256); walrus requires operands
    to be *produced* as float32r, so every matmul-feeding tile is f32r.
  * attention inner loop: per si-pair, 4 S^T matmuls -> one N=1024 exp ->
    8 attn@v matmuls accumulating into 4 per-t-subtile PSUM banks, emitted
    software-pipelined (S^T of pair p+1 before attn@v of pair p).
"""

import contextlib

import numpy as np

import concourse.bacc as bacc
import concourse.mybir as mybir
import concourse.tile as tile
from concourse.bass_utils import run_bass_kernel_spmd

dt = mybir.dt
AF = mybir.ActivationFunctionType

B, E, T, H, KW = 8, 256, 4096, 8, 3
NCORES = 8
P = 128                  # partitions / half of E
TCH = 512                # t-chunk width
NCH = T // TCH           # 8 chunks
NST = T // P             # 32 s-tiles
NSUB = TCH // P          # 4 t-subtiles per chunk
NPAIR = NST // 2         # 16 si-pairs
EA = E + 2               # vw width incl. ones column (padded even for fp32r)

TRACE = False
LAST = {}

_MODULE = None


def _build(tc, io):
    nc = tc.nc
    f32 = dt.float32
    f32r = dt.float32r
    x_d, wq_d, wk_d, wv_d, bq_d, bk_d, wf_d, be_d, oc_d, zc_d, y_d = io

    with contextlib.ExitStack() as ctx:
        const_p = ctx.enter_context(tc.tile_pool(name="const", bufs=1))
        x_p = ctx.enter_context(tc.tile_pool(name="xp", bufs=3))
        big_p = ctx.enter_context(tc.tile_pool(name="big", bufs=1))
        ch_p = ctx.enter_context(tc.tile_pool(name="ch", bufs=3))
        pt_p = ctx.enter_context(tc.tile_pool(name="ptp", bufs=4))
        out_p = ctx.enter_context(tc.tile_pool(name="outp", bufs=4))

        # x tiles for chunk 0 first so their DMAs lead the sync queue
        x0_tiles = {}
        for h in range(2):
            xt = x_p.tile([P, TCH + 2], f32r, tag=f"x0{h}", name=f"x0_{h}")
            eng = nc.sync if h == 0 else nc.gpsimd
            eng.dma_start(out=xt[:, 1 : TCH + 2], in_=x_d[h * P : (h + 1) * P, 0 : TCH + 1])
            eng.dma_start(out=xt[:, 0:1], in_=zc_d[:])
            x0_tiles[h] = xt

        # ---------------- constants ----------------
        w_sb = {}
        for pi, wd in ((1, wk_d), (2, wv_d), (0, wq_d)):
            for h in range(2):
                wt = const_p.tile([P, KW, P], f32r, tag=f"w{pi}{h}", name=f"w{pi}{h}")
                nc.sync.dma_start(out=wt[:], in_=wd[h])
                w_sb[pi, h] = wt
        bq_sb = const_p.tile([P, 2], f32, tag="bq", name="bq_sb")
        nc.gpsimd.dma_start(out=bq_sb[:], in_=bq_d[:])
        bk_sb = const_p.tile([P, 2], f32, tag="bk", name="bk_sb")
        nc.gpsimd.dma_start(out=bk_sb[:], in_=bk_d[:])
        wf_sb = []
        for h in range(2):
            wft = const_p.tile([P, E], f32r, tag=f"wf{h}", name=f"wf{h}")
            nc.gpsimd.dma_start(out=wft[:], in_=wf_d[h])
            wf_sb.append(wft)
        be_sb = const_p.tile([P, E], f32, tag="be", name="be_sb")
        nc.gpsimd.dma_start(out=be_sb[:], in_=be_d[:])

        # ---------------- resident tensors ----------------
        k_sb = []
        q_sb = []
        for h in range(2):
            kt = big_p.tile([P, T], f32r, tag=f"k{h}", name=f"k{h}")
            k_sb.append(kt)
            qt = big_p.tile([P, T], f32r, tag=f"q{h}", name=f"q{h}")
            q_sb.append(qt)
        vw_sb = big_p.tile([P, NST, EA], f32r, tag="vw", name="vw_sb")
        nc.gpsimd.dma_start(
            out=vw_sb[:, :, E:EA], in_=oc_d[:].rearrange("p (n o) -> p n o", o=2)
        )

        def load_x_chunk(tag, h, j):
            xt = x_p.tile([P, TCH + 2], f32r, tag=f"{tag}{h}", name=f"{tag}{h}")
            rows = slice(h * P, (h + 1) * P)
            c0 = j * TCH - 1
            if j == 0:
                nc.gpsimd.dma_start(out=xt[:, 0:1], in_=zc_d[:])
                nc.sync.dma_start(out=xt[:, 1 : TCH + 2], in_=x_d[rows, 0 : TCH + 1])
            elif j == NCH - 1:
                nc.gpsimd.dma_start(out=xt[:, TCH + 1 : TCH + 2], in_=zc_d[:])
                nc.sync.dma_start(out=xt[:, 0 : TCH + 1], in_=x_d[rows, c0:T])
            else:
                nc.sync.dma_start(out=xt[:], in_=x_d[rows, c0 : c0 + TCH + 2])
            return xt

        def conv_chunk(pool, ps_tag, w_key, xt):
            ps = pool.tile([P, TCH], f32, tag=ps_tag, name=f"ps_{ps_tag}")
            for kk in range(KW):
                nc.tensor.matmul(
                    ps[:],
                    w_sb[w_key][:, kk, :],
                    xt[:, kk : kk + TCH],
                    start=(kk == 0),
                    stop=(kk == KW - 1),
                )
            return ps

        # ---------------- phase 1: q, k, v -> vw' ----------------
        # chunk-paired: each conv weight tap is loaded once per two chunks
        with tc.tile_pool(name="ps_cv", bufs=2, space="PSUM") as ps_cv:
            for jp in range(NCH // 2):
                xts = {}
                for dj in range(2):
                    j = 2 * jp + dj
                    for h in range(2):
                        if j == 0 and h in x0_tiles:
                            xts[h, dj] = x0_tiles.pop(h)
                        else:
                            xts[h, dj] = load_x_chunk(f"x{dj}", h, j)
                v_ch = {}
                for h in range(2):
                    for pi in (1, 0, 2):
                        ps = {
                            dj: ps_cv.tile(
                                [P, TCH], f32, tag=f"cv{dj}", name=f"ps_cv{dj}"
                            )
                            for dj in range(2)
                        }
                        for kk in range(KW):
                            for dj in range(2):
                                nc.tensor.matmul(
                                    ps[dj][:],
                                    w_sb[pi, h][:, kk, :],
                                    xts[h, dj][:, kk : kk + TCH],
                                    start=(kk == 0),
                                    stop=(kk == KW - 1),
                                )
                        for dj in range(2):
                            j = 2 * jp + dj
                            tsl_c = slice(j * TCH, (j + 1) * TCH)
                            if pi == 1:
                                nc.vector.tensor_scalar_add(
                                    k_sb[h][:, tsl_c], ps[dj][:], bk_sb[:, h : h + 1]
                                )
                            elif pi == 0:
                                nc.vector.tensor_scalar_add(
                                    q_sb[h][:, tsl_c], ps[dj][:], bq_sb[:, h : h + 1]
                                )
                            else:
                                vt = ch_p.tile(
                                    [P, TCH], f32r, tag=f"vch{h}{dj}", name=f"vch{h}{dj}"
                                )
                                nc.vector.tensor_copy(vt[:], ps[dj][:])
                                v_ch[h, dj] = vt
                for dj in range(2):
                    j = 2 * jp + dj
                    for ti in range(NSUB):
                        si = j * NSUB + ti
                        ps_vw = ps_cv.tile([P, E], f32, tag="vwp", name="ps_vw")
                        tsl = slice(ti * P, (ti + 1) * P)
                        nc.tensor.matmul(
                            ps_vw[:],
                            v_ch[0, dj][:, tsl],
                            wf_sb[0][:],
                            start=True,
                            stop=False,
                        )
                        nc.tensor.matmul(
                            ps_vw[:],
                            v_ch[1, dj][:, tsl],
                            wf_sb[1][:],
                            start=False,
                            stop=True,
                        )
                        nc.vector.tensor_copy(vw_sb[:, si, 0:E], ps_vw[:])

        # ---------------- phase 2: attention ----------------
        with (
            tc.tile_pool(name="ps_st", bufs=2, space="PSUM") as ps_st,
            tc.tile_pool(name="ps_u", bufs=1, space="PSUM") as ps_u,
        ):
            for j in range(NCH):
                q_ch = [q_sb[h][:, j * TCH : (j + 1) * TCH] for h in range(2)]

                ups = [
                    ps_u.tile([P, EA], f32, tag=f"u{ti}", name=f"ups{ti}")
                    for ti in range(NSUB)
                ]

                def st_pair(p):
                    """S^T matmuls + one wide exp for si = 2p, 2p+1."""
                    ps = ps_st.tile([P, 2, TCH], f32, tag="st", name="ps_st")
                    pt = pt_p.tile([P, 2, TCH], f32r, tag="pt", name="pt")
                    for d in range(2):
                        ssl = slice((2 * p + d) * P, (2 * p + d + 1) * P)
                        nc.tensor.matmul(
                            ps[:, d, :],
                            k_sb[0][:, ssl],
                            q_ch[0][:],
                            start=True,
                            stop=False,
                        )
                        nc.tensor.matmul(
                            ps[:, d, :],
                            k_sb[1][:, ssl],
                            q_ch[1][:],
                            start=False,
                            stop=True,
                        )
                    nc.scalar.activation(pt[:], ps[:], AF.Exp)
                    return pt

                def u_pair(p, pt):
                    """attn@v for si = 2p, 2p+1 into the 4 subtile accums."""
                    for ti in range(NSUB):
                        for d in range(2):
                            si = 2 * p + d
                            nc.tensor.matmul(
                                ups[ti][:],
                                pt[:, d, ti * P : (ti + 1) * P],
                                vw_sb[:, si, :],
                                start=(si == 0),
                                stop=(si == NST - 1),
                            )

                prev = None
                for p in range(NPAIR):
                    pt = st_pair(p)
                    if prev is not None:
                        u_pair(p - 1, prev)
                    prev = pt
                u_pair(NPAIR - 1, prev)

                for ti in range(NSUB):
                    t0 = j * TCH + ti * P
                    rec = out_p.tile([P, 1], f32, tag="rec", name="rec")
                    nc.vector.reciprocal(rec[:], ups[ti][:, E : E + 1])
                    yt = out_p.tile([P, E], f32, tag="yt", name="yt")
                    nc.vector.scalar_tensor_tensor(
                        yt[:],
                        ups[ti][:, 0:E],
                        rec[:],
                        be_sb[:],
                        op0=mybir.AluOpType.mult,
                        op1=mybir.AluOpType.add,
                    )
                    nc.sync.dma_start(out=y_d[t0 : t0 + P, :], in_=yt[:])


def build_module():
    """Build + compile the Bass module (cached)."""
    global _MODULE
    if _MODULE is not None:
        return _MODULE
    nc = bacc.Bacc(
        "TRN2",
        target_bir_lowering=False,
        debug=False,
        enable_asserts=False,
        num_devices=NCORES,
    )
    f32 = dt.float32
    f32r = dt.float32r
    x_d = nc.dram_tensor("x", [E, T], f32r, kind="ExternalInput").ap()
    wq_d = nc.dram_tensor("wqb", [2, P, KW, P], f32r, kind="ExternalInput").ap()
    wk_d = nc.dram_tensor("wkb", [2, P, KW, P], f32r, kind="ExternalInput").ap()
    wv_d = nc.dram_tensor("wvb", [2, P, KW, P], f32r, kind="ExternalInput").ap()
    bq_d = nc.dram_tensor("bq2", [P, 2], f32, kind="ExternalInput").ap()
    bk_d = nc.dram_tensor("bk2", [P, 2], f32, kind="ExternalInput").ap()
    wf_d = nc.dram_tensor("wfcT", [2, P, E], f32r, kind="ExternalInput").ap()
    be_d = nc.dram_tensor("beff", [P, E], f32, kind="ExternalInput").ap()
    oc_d = nc.dram_tensor("onescol", [P, NST * 2], f32r, kind="ExternalInput").ap()
    zc_d = nc.dram_tensor("zcol", [P, 1], f32r, kind="ExternalInput").ap()
    y_d = nc.dram_tensor("y", [T, E], f32, kind="ExternalOutput").ap()

    with tile.TileContext(nc) as tc:
        _build(tc, (x_d, wq_d, wk_d, wv_d, bq_d, bk_d, wf_d, be_d, oc_d, zc_d, y_d))
    nc.compile()
    _MODULE = nc
    return nc


def _marshal(x, wq, bq, wk, bk, wv, bv, w_fc, b_fc):
    """Host-side input prep (weights only -- all tiny)."""
    scale = np.float32(1.0 / np.sqrt(E))

    def blockdiag(w):
        # w: [E, E//H, KW] grouped conv weight ->
        # out[h, in_local, kk, out_local] block-diagonal per half.
        out = np.zeros((2, P, KW, P), np.float32)
        gs = E // H  # 32
        for h in range(2):
            for g in range(4):
                grp = 4 * h + g
                blk = w[gs * grp : gs * (grp + 1), :, :]  # [out c', in i, kk]
                for kk in range(KW):
                    out[h, gs * g : gs * (g + 1), kk, gs * g : gs * (g + 1)] = blk[
                        :, :, kk
                    ].T
        return out

    wqb = blockdiag(wq) * scale
    wkb = blockdiag(wk)
    wvb = blockdiag(wv)
    bq2 = np.ascontiguousarray((bq * scale).reshape(2, P).T)
    bk2 = np.ascontiguousarray(bk.reshape(2, P).T)
    wfcT = np.ascontiguousarray(w_fc.T.reshape(2, P, E))
    beff = np.ascontiguousarray(
        np.broadcast_to((w_fc @ bv + b_fc).reshape(1, E), (P, E))
    )
    return {
        "wqb": np.ascontiguousarray(wqb),
        "wkb": np.ascontiguousarray(wkb),
        "wvb": np.ascontiguousarray(wvb),
        "bq2": bq2,
        "bk2": bk2,
        "wfcT": wfcT,
        "beff": beff,
        "onescol": np.ones((P, NST * 2), np.float32),
        "zcol": np.zeros((P, 1), np.float32),
    }


def kernel(x, wq, bq, wk, bk, wv, bv, w_fc, b_fc, num_heads):
    x = np.asarray(x, np.float32)
    consts = _marshal(
        x,
        np.asarray(wq, np.float32),
        np.asarray(bq, np.float32),
        np.asarray(wk, np.float32),
        np.asarray(bk, np.float32),
        np.asarray(wv, np.float32),
        np.asarray(bv, np.float32),
        np.asarray(w_fc, np.float32),
        np.asarray(b_fc, np.float32),
    )
    nc = build_module()
    in_maps = [{"x": np.ascontiguousarray(x[b]), **consts} for b in range(B)]
    res = run_bass_kernel_spmd(nc, in_maps, core_ids=list(range(NCORES)), trace=TRACE)
    LAST["exec_time_ns"] = res.exec_time_ns
    LAST["mean_exec_time_ns"] = res.mean_exec_time_ns
    LAST["results"] = res
    out = np.stack([res.results[b]["y"] for b in range(B)], axis=0)
    return out



# revision 12
# speedup vs baseline: 1.0348x; 1.0348x over previous
"""ConvAttention kernel for 8x TRN2 NeuronCores.

Model (per batch item b):
    q/k/v = grouped_conv1d(x_b, w, b, groups=8)        # [E, T] -> [E, T]
    S     = (q^T k) / sqrt(E)                          # [T, T]
    P     = softmax(S, axis=-1)
    y     = (P @ v^T) @ w_fc^T + b_fc                  # [T, E]

Sharding: pure data-parallel over batch B=8 -> 8 cores, weights replicated.

Per-core algorithm (v2: fp8 DoubleRow on the T^2 matmuls):
  * conv projections as block-diagonal [128,128] fp32r matmuls per tap; q/k are
    written as fp8e4 in a DoubleRow-packed layout [128, 2, T] (the two
    128-channel halves on the middle axis).  q/k stay unit-variance (the
    1/sqrt(E) score scale is folded into the exp activation's `scale`).
  * vw' = v^T @ w_fc^T + beff (fc folded into v, conv bias bv + b_fc folded
    into beff which is added to every vw row: u' = u + r*beff, so
    u'/r = u/r + beff).  vw' stored fp8e4 [128, NST, 272].
  * S^T tiles per si-pair with ONE DoubleRow matmul per si (contraction 256 =
    both halves at once, 2 MACs/cell/cycle).
  * exp(S_raw/16 - 2) on the scalar engine straight to fp8e4 (the -2 shift
    guards fp8 overflow and cancels in the normalization).
  * attn@v runs TRANSPOSED with vw' as the DoubleRow stationary and the exp'd
    P^T pair tile as the moving operand: u'^T[e,t] accumulates over all 32
    si into 2 PSUM banks; a third M=1 DoubleRow matmul with a ones stationary
    accumulates the row sums r[t].
  * normalize: rinv = 1/r on DVE, partition-broadcast on GpSimd, y^T = u'^T *
    rinv on DVE, DMA'd out as y^T [2,128,T]; the host un-transposes.
"""

import contextlib

import numpy as np

import concourse.bacc as bacc
import concourse.mybir as mybir
import concourse.tile as tile
from concourse.bass_utils import run_bass_kernel_spmd

dt = mybir.dt
AF = mybir.ActivationFunctionType
DR = mybir.MatmulPerfMode.DoubleRow

B, E, T, H, KW = 8, 256, 4096, 8, 3
NCORES = 8
P = 128                  # partitions / half of E
TCH = 512                # t-chunk width
NCH = T // TCH           # 8 chunks
NST = T // P             # 32 s-tiles
NPAIR = NST // 2         # 16 si-pairs
VWW = 272                # vw row stride (>=E, multiple of 16 for DR ldweights)
SCALE = float(1.0 / np.sqrt(E))   # folded into the exp activation
SHIFT = -2.0             # exp shift; cancels in normalization, avoids fp8 ovf

TRACE = False
LAST = {}

_MODULE = None


def _build(tc, io):
    nc = tc.nc
    f32 = dt.float32
    f32r = dt.float32r
    f8 = dt.float8e4
    f16 = dt.float16
    x_d, wq_d, wk_d, wv_d, bq_d, bk_d, wf_d, be_d, zc_d, y_d = io
    wd_map = {0: wq_d, 1: wk_d, 2: wv_d}

    with contextlib.ExitStack() as ctx:
        const_p = ctx.enter_context(tc.tile_pool(name="const", bufs=1))
        x_p = ctx.enter_context(tc.tile_pool(name="xp", bufs=3))
        big_p = ctx.enter_context(tc.tile_pool(name="big", bufs=1))
        ch_p = ctx.enter_context(tc.tile_pool(name="ch", bufs=3))
        pt_p = ctx.enter_context(tc.tile_pool(name="ptp", bufs=4))
        out_p = ctx.enter_context(tc.tile_pool(name="outp", bufs=2))

        # x tiles for chunk 0 first so their DMAs lead the sync queue
        x0_tiles = {}
        for h in range(2):
            xt = x_p.tile([P, TCH + 2], f32r, tag=f"x0{h}", name=f"x0_{h}")
            eng = nc.sync if h == 0 else nc.gpsimd
            eng.dma_start(out=xt[:, 1 : TCH + 2], in_=x_d[h * P : (h + 1) * P, 0 : TCH + 1])
            eng.dma_start(out=xt[:, 0:1], in_=zc_d[:])
            x0_tiles[h] = xt

        # ---------------- constants ----------------
        # conv weights in use-order (h-major), interleaved across both queues so
        # the first conv groups' operands arrive earliest; fc weights (not
        # needed until the vw prepass) load last.
        w_sb = {}
        for h in range(2):
            for pi in (1, 0, 2):
                wt = const_p.tile([P, KW, P], f32r, tag=f"w{pi}{h}", name=f"w{pi}{h}")
                eng = nc.sync if h == 0 else nc.gpsimd
                eng.dma_start(out=wt[:], in_=wd_map[pi][h])
                w_sb[pi, h] = wt
        bq_sb = const_p.tile([P, 2], f32, tag="bq", name="bq_sb")
        nc.gpsimd.dma_start(out=bq_sb[:], in_=bq_d[:])
        bk_sb = const_p.tile([P, 2], f32, tag="bk", name="bk_sb")
        nc.gpsimd.dma_start(out=bk_sb[:], in_=bk_d[:])
        wf_sb = []
        for h in range(2):
            wft = const_p.tile([P, E], f32r, tag=f"wf{h}", name=f"wf{h}")
            nc.gpsimd.dma_start(out=wft[:], in_=wf_d[h])
            wf_sb.append(wft)
        be_sb = const_p.tile([P, E], f32, tag="be", name="be_sb")
        nc.gpsimd.dma_start(out=be_sb[:], in_=be_d[:])

        # fp8 ones stationary for the row-sum matmul: [P, 2, 16], use [:, :, 0:1]
        ones_f = const_p.tile([P, 2], f32, tag="onef", name="ones_f")
        nc.vector.memset(ones_f[:], 1.0)
        # per-partition exp shift (bias operand for the activation)
        shift_sb = const_p.tile([P, 1], f32, tag="shift", name="shift_sb")
        nc.vector.memset(shift_sb[:], SHIFT)
        ones_pk = const_p.tile([P, 2, 16], f8, tag="onep", name="ones_pk")
        for i in range(2):
            nc.vector.tensor_copy(ones_pk[:, i, 0:1], ones_f[:, i : i + 1])

        # ---------------- resident tensors ----------------
        # q/k in fp16: fp8 scores dominate the error budget; fp16 streams at
        # the same 1 col/cycle as bf16 with 4 more mantissa bits.
        q_pk = big_p.tile([P, 2, T], f16, tag="qpk", name="q_pk")
        k_pk = big_p.tile([P, 2, T], f16, tag="kpk", name="k_pk")
        vw_sb = big_p.tile([P, NST, VWW], f8, tag="vw", name="vw_sb")

        def load_x_chunk(tag, h, j):
            xt = x_p.tile([P, TCH + 2], f32r, tag=f"{tag}{h}", name=f"{tag}{h}")
            rows = slice(h * P, (h + 1) * P)
            c0 = j * TCH - 1
            if j == 0:
                nc.gpsimd.dma_start(out=xt[:, 0:1], in_=zc_d[:])
                nc.sync.dma_start(out=xt[:, 1 : TCH + 2], in_=x_d[rows, 0 : TCH + 1])
            elif j == NCH - 1:
                nc.gpsimd.dma_start(out=xt[:, TCH + 1 : TCH + 2], in_=zc_d[:])
                nc.sync.dma_start(out=xt[:, 0 : TCH + 1], in_=x_d[rows, c0:T])
            else:
                nc.sync.dma_start(out=xt[:], in_=x_d[rows, c0 : c0 + TCH + 2])
            return xt

        # ---------------- phase 1: q, k, v -> vw' ----------------
        # chunk-paired: each conv weight tap is loaded once per two chunks
        with tc.tile_pool(name="ps_cv", bufs=2, space="PSUM") as ps_cv:
            for jp in range(NCH // 2):
                xts = {}
                for dj in range(2):
                    j = 2 * jp + dj
                    for h in range(2):
                        if j == 0 and h in x0_tiles:
                            xts[h, dj] = x0_tiles.pop(h)
                        else:
                            xts[h, dj] = load_x_chunk(f"x{dj}", h, j)
                v_ch = {}
                for h in range(2):
                    for pi in (1, 0, 2):
                        ps = {
                            dj: ps_cv.tile(
                                [P, TCH], f32, tag=f"cv{dj}", name=f"ps_cv{dj}"
                            )
                            for dj in range(2)
                        }
                        for kk in range(KW):
                            for dj in range(2):
                                nc.tensor.matmul(
                                    ps[dj][:],
                                    w_sb[pi, h][:, kk, :],
                                    xts[h, dj][:, kk : kk + TCH],
                                    start=(kk == 0),
                                    stop=(kk == KW - 1),
                                )
                        for dj in range(2):
                            j = 2 * jp + dj
                            tsl_c = slice(j * TCH, (j + 1) * TCH)
                            if pi == 1:
                                nc.vector.tensor_scalar_add(
                                    k_pk[:, h, tsl_c], ps[dj][:], bk_sb[:, h : h + 1]
                                )
                            elif pi == 0:
                                nc.vector.tensor_scalar_add(
                                    q_pk[:, h, tsl_c], ps[dj][:], bq_sb[:, h : h + 1]
                                )
                            else:
                                vt = ch_p.tile(
                                    [P, TCH], f32r, tag=f"vch{h}{dj}", name=f"vch{h}{dj}"
                                )
                                nc.vector.tensor_copy(vt[:], ps[dj][:])
                                v_ch[h, dj] = vt
                for dj in range(2):
                    j = 2 * jp + dj
                    for ti in range(TCH // P):
                        si = j * (TCH // P) + ti
                        ps_vw = ps_cv.tile([P, E], f32, tag="vwp", name="ps_vw")
                        tsl = slice(ti * P, (ti + 1) * P)
                        nc.tensor.matmul(
                            ps_vw[:],
                            v_ch[0, dj][:, tsl],
                            wf_sb[0][:],
                            start=True,
                            stop=False,
                        )
                        nc.tensor.matmul(
                            ps_vw[:],
                            v_ch[1, dj][:, tsl],
                            wf_sb[1][:],
                            start=False,
                            stop=True,
                        )
                        # vw' = v@wfc^T + beff row-fold, cast to fp8
                        nc.vector.tensor_tensor(
                            vw_sb[:, si, 0:E], ps_vw[:], be_sb[:], mybir.AluOpType.add
                        )

        # ---------------- phase 2: attention ----------------
        with (
            tc.tile_pool(name="ps_st", bufs=2, space="PSUM") as ps_st,
            tc.tile_pool(name="ps_u", bufs=1, space="PSUM") as ps_u,
        ):
            for j in range(NCH):
                tslj = slice(j * TCH, (j + 1) * TCH)

                ups = [
                    ps_u.tile([P, TCH], f32, tag=f"u{h}", name=f"ups{h}")
                    for h in range(2)
                ]
                rps = ps_u.tile([P, TCH], f32, tag="r", name="rps", bufs=2)

                def st_pair(p):
                    """S^T fp16 matmuls + one wide exp for si = 2p, 2p+1."""
                    ps = ps_st.tile([P, 2, TCH], f32, tag="st", name="ps_stt")
                    pt = pt_p.tile([P, 2, TCH], f8, tag="pt", name="pt")
                    for d in range(2):
                        ssl = slice((2 * p + d) * P, (2 * p + d + 1) * P)
                        for h in range(2):
                            nc.tensor.matmul(
                                ps[:, d, :],
                                k_pk[:, h, ssl],
                                q_pk[:, h, tslj],
                                start=(h == 0),
                                stop=(h == 1),
                            )
                    nc.scalar.activation(
                        pt[:], ps[:], AF.Exp, bias=shift_sb[:], scale=SCALE
                    )
                    return pt

                def u_pair(p, pt):
                    """u'^T += vw'^T-DR @ P^T-pair; r += ones-DR @ P^T-pair."""
                    first, last = (p == 0), (p == NPAIR - 1)
                    for h in range(2):
                        nc.tensor.matmul(
                            ups[h][:],
                            vw_sb[:, 2 * p : 2 * p + 2, h * P : (h + 1) * P],
                            pt[:],
                            start=first,
                            stop=last,
                            perf_mode=DR,
                        )
                    nc.tensor.matmul(
                        rps[0:1, :],
                        ones_pk[:, :, 0:1],
                        pt[:],
                        start=first,
                        stop=last,
                        perf_mode=DR,
                    )

                prev = None
                for p in range(NPAIR):
                    pt = st_pair(p)
                    if prev is not None:
                        u_pair(p - 1, prev)
                    prev = pt
                u_pair(NPAIR - 1, prev)

                # broadcast r to all partitions FIRST, then reciprocal runs on
                # all 128 DVE lanes (a [1,512] reciprocal is 6x slower);
                # gpsimd can't read PSUM, so hop through SBUF
                rcp = out_p.tile([1, TCH], f32, tag="rcp", name="rcp")
                nc.vector.tensor_copy(rcp[:], rps[0:1, :])
                rbr = out_p.tile([P, TCH], f32, tag="rbr", name="rbr")
                nc.gpsimd.partition_broadcast(rbr[:], rcp[:])
                rb = out_p.tile([P, TCH], f32, tag="rb", name="rb")
                nc.vector.reciprocal(rb[:], rbr[:])
                for h in range(2):
                    yt = out_p.tile([P, TCH], f32, tag=f"yt{h}", name=f"yt{h}")
                    nc.vector.tensor_tensor(
                        yt[:], ups[h][:], rb[:], mybir.AluOpType.mult
                    )
                    nc.sync.dma_start(out=y_d[h][:, tslj], in_=yt[:])


def build_module():
    """Build + compile the Bass module (cached)."""
    global _MODULE
    if _MODULE is not None:
        return _MODULE
    nc = bacc.Bacc(
        "TRN2",
        target_bir_lowering=False,
        debug=False,
        enable_asserts=False,
        num_devices=NCORES,
    )
    f32 = dt.float32
    f32r = dt.float32r
    x_d = nc.dram_tensor("x", [E, T], f32r, kind="ExternalInput").ap()
    wq_d = nc.dram_tensor("wqb", [2, P, KW, P], f32r, kind="ExternalInput").ap()
    wk_d = nc.dram_tensor("wkb", [2, P, KW, P], f32r, kind="ExternalInput").ap()
    wv_d = nc.dram_tensor("wvb", [2, P, KW, P], f32r, kind="ExternalInput").ap()
    bq_d = nc.dram_tensor("bq2", [P, 2], f32, kind="ExternalInput").ap()
    bk_d = nc.dram_tensor("bk2", [P, 2], f32, kind="ExternalInput").ap()
    wf_d = nc.dram_tensor("wfcT", [2, P, E], f32r, kind="ExternalInput").ap()
    be_d = nc.dram_tensor("beff", [P, E], f32, kind="ExternalInput").ap()
    zc_d = nc.dram_tensor("zcol", [P, 1], f32r, kind="ExternalInput").ap()
    y_d = nc.dram_tensor("yT", [2, P, T], f32, kind="ExternalOutput").ap()

    with tile.TileContext(nc) as tc:
        _build(tc, (x_d, wq_d, wk_d, wv_d, bq_d, bk_d, wf_d, be_d, zc_d, y_d))
    nc.compile()
    _MODULE = nc
    return nc


def _marshal(x, wq, bq, wk, bk, wv, bv, w_fc, b_fc):
    """Host-side input prep (weights only -- all tiny)."""

    def blockdiag(w):
        # w: [E, E//H, KW] grouped conv weight ->
        # out[h, in_local, kk, out_local] block-diagonal per half.
        out = np.zeros((2, P, KW, P), np.float32)
        gs = E // H  # 32
        for h in range(2):
            for g in range(4):
                grp = 4 * h + g
                blk = w[gs * grp : gs * (grp + 1), :, :]  # [out c', in i, kk]
                for kk in range(KW):
                    out[h, gs * g : gs * (g + 1), kk, gs * g : gs * (g + 1)] = blk[
                        :, :, kk
                    ].T
        return out

    wqb = blockdiag(wq)
    wkb = blockdiag(wk)
    wvb = blockdiag(wv)
    bq2 = np.ascontiguousarray(bq.reshape(2, P).T)
    bk2 = np.ascontiguousarray(bk.reshape(2, P).T)
    wfcT = np.ascontiguousarray(w_fc.T.reshape(2, P, E))
    beff = np.ascontiguousarray(
        np.broadcast_to((w_fc @ bv + b_fc).reshape(1, E), (P, E))
    )
    return {
        "wqb": np.ascontiguousarray(wqb),
        "wkb": np.ascontiguousarray(wkb),
        "wvb": np.ascontiguousarray(wvb),
        "bq2": bq2,
        "bk2": bk2,
        "wfcT": wfcT,
        "beff": beff,
        "zcol": np.zeros((P, 1), np.float32),
    }


def kernel(x, wq, bq, wk, bk, wv, bv, w_fc, b_fc, num_heads):
    x = np.asarray(x, np.float32)
    consts = _marshal(
        x,
        np.asarray(wq, np.float32),
        np.asarray(bq, np.float32),
        np.asarray(wk, np.float32),
        np.asarray(bk, np.float32),
        np.asarray(wv, np.float32),
        np.asarray(bv, np.float32),
        np.asarray(w_fc, np.float32),
        np.asarray(b_fc, np.float32),
    )
    nc = build_module()
    in_maps = [{"x": np.ascontiguousarray(x[b]), **consts} for b in range(B)]
    res = run_bass_kernel_spmd(nc, in_maps, core_ids=list(range(NCORES)), trace=TRACE)
    LAST["exec_time_ns"] = res.exec_time_ns
    LAST["mean_exec_time_ns"] = res.mean_exec_time_ns
    LAST["results"] = res
    out = np.stack(
        [
            np.ascontiguousarray(res.results[b]["yT"].reshape(E, T).T)
            for b in range(B)
        ],
        axis=0,
    )
    return out


# revision 20
# speedup vs baseline: 1.1346x; 1.0964x over previous
"""ConvAttention kernel for 8x TRN2 NeuronCores.

Model (per batch item b):
    q/k/v = grouped_conv1d(x_b, w, b, groups=8)        # [E, T] -> [E, T]
    S     = (q^T k) / sqrt(E)                          # [T, T]
    P     = softmax(S, axis=-1)
    y     = (P @ v^T) @ w_fc^T + b_fc                  # [T, E]

Sharding: pure data-parallel over batch B=8 -> 8 cores, weights replicated.

Per-core algorithm (v2: fp8 DoubleRow on the T^2 matmuls):
  * conv projections as block-diagonal [128,128] fp32r matmuls per tap; q/k are
    written as fp8e4 in a DoubleRow-packed layout [128, 2, T] (the two
    128-channel halves on the middle axis).  q/k stay unit-variance (the
    1/sqrt(E) score scale is folded into the exp activation's `scale`).
  * vw' = v^T @ w_fc^T + beff (fc folded into v, conv bias bv + b_fc folded
    into beff which is added to every vw row: u' = u + r*beff, so
    u'/r = u/r + beff).  vw' stored fp8e4 [128, NST, 272].
  * S^T tiles per si-pair with ONE DoubleRow matmul per si (contraction 256 =
    both halves at once, 2 MACs/cell/cycle).
  * exp(S_raw/16 - 2) on the scalar engine straight to fp8e4 (the -2 shift
    guards fp8 overflow and cancels in the normalization).
  * attn@v runs TRANSPOSED with vw' as the DoubleRow stationary and the exp'd
    P^T pair tile as the moving operand: u'^T[e,t] accumulates over all 32
    si into 2 PSUM banks; a third M=1 DoubleRow matmul with a ones stationary
    accumulates the row sums r[t].
  * normalize: rinv = 1/r on DVE, partition-broadcast on GpSimd, y^T = u'^T *
    rinv on DVE, DMA'd out as y^T [2,128,T]; the host un-transposes.
"""

import contextlib

import numpy as np

import concourse.bacc as bacc
import concourse.mybir as mybir
import concourse.tile as tile
from concourse.bass_utils import run_bass_kernel_spmd

dt = mybir.dt
AF = mybir.ActivationFunctionType
DR = mybir.MatmulPerfMode.DoubleRow

B, E, T, H, KW = 8, 256, 4096, 8, 3
NCORES = 8
P = 128                  # partitions / half of E
TCH = 512                # t-chunk width
NCH = T // TCH           # 8 chunks
NST = T // P             # 32 s-tiles
NPAIR = NST // 2         # 16 si-pairs
VWW = 272                # vw row stride (>=E, multiple of 16 for DR ldweights)
SCALE = float(1.0 / np.sqrt(E))   # folded into the exp activation
SHIFT = -2.0             # exp shift; cancels in normalization, avoids fp8 ovf

TRACE = False
LAST = {}

_MODULE = None


def _build(tc, io):
    nc = tc.nc
    f32 = dt.float32
    f32r = dt.float32r
    f8 = dt.float8e4
    f16 = dt.float16
    x_d, wq_d, wk_d, wv_d, bq_d, bk_d, wf_d, be_d, zc_d, y_d = io
    wd_map = {0: wq_d, 1: wk_d, 2: wv_d}

    with contextlib.ExitStack() as ctx:
        const_p = ctx.enter_context(tc.tile_pool(name="const", bufs=1))
        x_p = ctx.enter_context(tc.tile_pool(name="xp", bufs=3))
        big_p = ctx.enter_context(tc.tile_pool(name="big", bufs=1))
        ch_p = ctx.enter_context(tc.tile_pool(name="ch", bufs=3))
        pt_p = ctx.enter_context(tc.tile_pool(name="ptp", bufs=4))
        out_p = ctx.enter_context(tc.tile_pool(name="outp", bufs=2))

        # Startup DMAs spread over 4 engine queues in need-order: the first
        # conv group (k, h=0, chunks 0+1) needs w(1,0) + x(j0,h0) + x(j1,h0).
        x0_tiles = {}
        xt = x_p.tile([P, TCH + 2], f32r, tag="x00", name="x0_0")
        nc.sync.dma_start(out=xt[:, 1 : TCH + 2], in_=x_d[0:P, 0 : TCH + 1])
        nc.sync.dma_start(out=xt[:, 0:1], in_=zc_d[:])
        x0_tiles[0] = xt
        xt = x_p.tile([P, TCH + 2], f32r, tag="x01", name="x0_1")
        nc.scalar.dma_start(out=xt[:, 1 : TCH + 2], in_=x_d[P : 2 * P, 0 : TCH + 1])
        nc.scalar.dma_start(out=xt[:, 0:1], in_=zc_d[:])
        x0_tiles[1] = xt

        # ---------------- constants ----------------
        bq_sb = const_p.tile([P, 2], f32, tag="bq", name="bq_sb")
        nc.gpsimd.dma_start(out=bq_sb[:], in_=bq_d[:])
        bk_sb = const_p.tile([P, 2], f32, tag="bk", name="bk_sb")
        nc.gpsimd.dma_start(out=bk_sb[:], in_=bk_d[:])
        w_sb = {}
        w_eng = {(1, 0): nc.sync, (1, 1): nc.scalar,
                 (0, 0): nc.gpsimd, (0, 1): nc.gpsimd,
                 (2, 0): nc.gpsimd, (2, 1): nc.gpsimd}
        for h in range(2):
            for pi in (1, 0, 2):
                wt = const_p.tile([P, KW, P], f32r, tag=f"w{pi}{h}", name=f"w{pi}{h}")
                w_eng[pi, h].dma_start(out=wt[:], in_=wd_map[pi][h])
                w_sb[pi, h] = wt
        wf_sb = []
        for h in range(2):
            wft = const_p.tile([P, E], f32r, tag=f"wf{h}", name=f"wf{h}")
            nc.gpsimd.dma_start(out=wft[:], in_=wf_d[h])
            wf_sb.append(wft)
        be_sb = const_p.tile([P, E], f32, tag="be", name="be_sb")
        nc.gpsimd.dma_start(out=be_sb[:], in_=be_d[:])

        # fp8 ones stationary for the row-sum matmul: [P, 2, 16], use [:, :, 0:1]
        ones_f = const_p.tile([P, 2], f32, tag="onef", name="ones_f")
        nc.vector.memset(ones_f[:], 1.0)
        # per-partition exp shift (bias operand for the activation)
        shift_sb = const_p.tile([P, 1], f32, tag="shift", name="shift_sb")
        nc.vector.memset(shift_sb[:], SHIFT)
        ones_pk = const_p.tile([P, 2, 16], f8, tag="onep", name="ones_pk")
        for i in range(2):
            nc.vector.tensor_copy(ones_pk[:, i, 0:1], ones_f[:, i : i + 1])

        # ---------------- resident tensors ----------------
        # q/k in fp16: fp8 scores dominate the error budget; fp16 streams at
        # the same 1 col/cycle as bf16 with 4 more mantissa bits.
        q_pk = big_p.tile([P, 2, T], f16, tag="qpk", name="q_pk")
        k_pk = big_p.tile([P, 2, T], f16, tag="kpk", name="k_pk")
        vw_sb = big_p.tile([P, NST, VWW], f8, tag="vw", name="vw_sb")

        def load_x_chunk(tag, h, j):
            xt = x_p.tile([P, TCH + 2], f32r, tag=f"{tag}{h}", name=f"{tag}{h}")
            rows = slice(h * P, (h + 1) * P)
            c0 = j * TCH - 1
            eng = nc.sync if h == 0 else nc.scalar
            if j == 0:
                eng.dma_start(out=xt[:, 0:1], in_=zc_d[:])
                eng.dma_start(out=xt[:, 1 : TCH + 2], in_=x_d[rows, 0 : TCH + 1])
            elif j == NCH - 1:
                eng.dma_start(out=xt[:, TCH + 1 : TCH + 2], in_=zc_d[:])
                eng.dma_start(out=xt[:, 0 : TCH + 1], in_=x_d[rows, c0:T])
            else:
                eng.dma_start(out=xt[:], in_=x_d[rows, c0 : c0 + TCH + 2])
            return xt

        # ---------------- phase 1: q, k, v -> vw' ----------------
        # chunk-paired: each conv weight tap is loaded once per two chunks
        with tc.tile_pool(name="ps_cv", bufs=2, space="PSUM") as ps_cv:
            for jp in range(NCH // 2):
                xts = {}
                for h in range(2):
                    for dj in range(2):
                        j = 2 * jp + dj
                        if j == 0 and h in x0_tiles:
                            xts[h, dj] = x0_tiles.pop(h)
                        else:
                            xts[h, dj] = load_x_chunk(f"x{dj}", h, j)
                v_ch = {}
                for h in range(2):
                    for pi in (1, 0, 2):
                        ps = {
                            dj: ps_cv.tile(
                                [P, TCH], f32, tag=f"cv{dj}", name=f"ps_cv{dj}"
                            )
                            for dj in range(2)
                        }
                        for kk in range(KW):
                            for dj in range(2):
                                nc.tensor.matmul(
                                    ps[dj][:],
                                    w_sb[pi, h][:, kk, :],
                                    xts[h, dj][:, kk : kk + TCH],
                                    start=(kk == 0),
                                    stop=(kk == KW - 1),
                                )
                        for dj in range(2):
                            j = 2 * jp + dj
                            tsl_c = slice(j * TCH, (j + 1) * TCH)
                            if pi == 1:
                                nc.vector.tensor_scalar_add(
                                    k_pk[:, h, tsl_c], ps[dj][:], bk_sb[:, h : h + 1]
                                )
                            elif pi == 0:
                                nc.vector.tensor_scalar_add(
                                    q_pk[:, h, tsl_c], ps[dj][:], bq_sb[:, h : h + 1]
                                )
                            else:
                                vt = ch_p.tile(
                                    [P, TCH], f32r, tag=f"vch{h}{dj}", name=f"vch{h}{dj}"
                                )
                                nc.vector.tensor_copy(vt[:], ps[dj][:])
                                v_ch[h, dj] = vt
                for dj in range(2):
                    j = 2 * jp + dj
                    for ti in range(TCH // P):
                        si = j * (TCH // P) + ti
                        ps_vw = ps_cv.tile([P, E], f32, tag="vwp", name="ps_vw")
                        tsl = slice(ti * P, (ti + 1) * P)
                        nc.tensor.matmul(
                            ps_vw[:],
                            v_ch[0, dj][:, tsl],
                            wf_sb[0][:],
                            start=True,
                            stop=False,
                        )
                        nc.tensor.matmul(
                            ps_vw[:],
                            v_ch[1, dj][:, tsl],
                            wf_sb[1][:],
                            start=False,
                            stop=True,
                        )
                        # vw' = v@wfc^T + beff row-fold, cast to fp8
                        nc.vector.tensor_tensor(
                            vw_sb[:, si, 0:E], ps_vw[:], be_sb[:], mybir.AluOpType.add
                        )

        # ---------------- phase 2: attention ----------------
        # PSUM: st 2x1 + u 2x2 + r 2x1 = 8 banks.  ups/r double-buffered so
        # the normalize chain of chunk j overlaps chunk j+1's matmuls.
        with (
            tc.tile_pool(name="ps_st", bufs=2, space="PSUM") as ps_st,
            tc.tile_pool(name="ps_u", bufs=2, space="PSUM") as ps_u,
        ):
            for j in range(NCH):
                tslj = slice(j * TCH, (j + 1) * TCH)

                ups = [
                    ps_u.tile([P, TCH], f32, tag=f"u{h}", name=f"ups{h}")
                    for h in range(2)
                ]
                rps = ps_u.tile([P, TCH], f32, tag="r", name="rps")

                def st_d(p, d, pt):
                    """S^T fp16 matmuls + exp for si = 2p+d into pt plane d."""
                    ps = ps_st.tile([P, TCH], f32, tag="st", name="ps_stt")
                    ssl = slice((2 * p + d) * P, (2 * p + d + 1) * P)
                    for h in range(2):
                        nc.tensor.matmul(
                            ps[:],
                            k_pk[:, h, ssl],
                            q_pk[:, h, tslj],
                            start=(h == 0),
                            stop=(h == 1),
                        )
                    nc.scalar.activation(
                        pt[:, d, :], ps[:], AF.Exp, bias=shift_sb[:], scale=SCALE
                    )

                def u_h(p, pt, h):
                    nc.tensor.matmul(
                        ups[h][:],
                        vw_sb[:, 2 * p : 2 * p + 2, h * P : (h + 1) * P],
                        pt[:],
                        start=(p == 0),
                        stop=(p == NPAIR - 1),
                        perf_mode=DR,
                    )

                def u_r(p, pt):
                    nc.tensor.matmul(
                        rps[0:1, :],
                        ones_pk[:, :, 0:1],
                        pt[:],
                        start=(p == 0),
                        stop=(p == NPAIR - 1),
                        perf_mode=DR,
                    )

                # software pipeline, DR matmuls interleaved between fp16
                # score matmuls so their double-width LDWEIGHTS hide under
                # the fp16 streams
                prev = None
                for p in range(NPAIR):
                    pt = pt_p.tile([P, 2, TCH], f8, tag="pt", name="pt")
                    st_d(p, 0, pt)
                    if prev is not None:
                        u_h(p - 1, prev, 0)
                    st_d(p, 1, pt)
                    if prev is not None:
                        u_h(p - 1, prev, 1)
                        u_r(p - 1, prev)
                    prev = pt
                u_h(NPAIR - 1, prev, 0)
                u_h(NPAIR - 1, prev, 1)
                u_r(NPAIR - 1, prev)

                # normalize: r -> SBUF -> broadcast (gpsimd) -> 1/r on all 128
                # DVE lanes (approx_fast: ~4e-6 rel, 5x faster than exact)
                rcp = out_p.tile([1, TCH], f32, tag="rcp", name="rcp")
                nc.vector.tensor_copy(rcp[:], rps[0:1, :])
                rbr = out_p.tile([P, TCH], f32, tag="rbr", name="rbr")
                nc.gpsimd.partition_broadcast(rbr[:], rcp[:])
                rb = out_p.tile([P, TCH], f32, tag="rb", name="rb")
                nc.vector.reciprocal_approx_fast(rb[:], rbr[:])
                for h in range(2):
                    yt = out_p.tile([P, TCH], f32, tag=f"yt{h}", name=f"yt{h}")
                    nc.vector.tensor_tensor(
                        yt[:], ups[h][:], rb[:], mybir.AluOpType.mult
                    )
                    nc.sync.dma_start(out=y_d[h][:, tslj], in_=yt[:])


def build_module():
    """Build + compile the Bass module (cached)."""
    global _MODULE
    if _MODULE is not None:
        return _MODULE
    nc = bacc.Bacc(
        "TRN2",
        target_bir_lowering=False,
        debug=False,
        enable_asserts=False,
        num_devices=NCORES,
    )
    f32 = dt.float32
    f32r = dt.float32r
    x_d = nc.dram_tensor("x", [E, T], f32r, kind="ExternalInput").ap()
    wq_d = nc.dram_tensor("wqb", [2, P, KW, P], f32r, kind="ExternalInput").ap()
    wk_d = nc.dram_tensor("wkb", [2, P, KW, P], f32r, kind="ExternalInput").ap()
    wv_d = nc.dram_tensor("wvb", [2, P, KW, P], f32r, kind="ExternalInput").ap()
    bq_d = nc.dram_tensor("bq2", [P, 2], f32, kind="ExternalInput").ap()
    bk_d = nc.dram_tensor("bk2", [P, 2], f32, kind="ExternalInput").ap()
    wf_d = nc.dram_tensor("wfcT", [2, P, E], f32r, kind="ExternalInput").ap()
    be_d = nc.dram_tensor("beff", [P, E], f32, kind="ExternalInput").ap()
    zc_d = nc.dram_tensor("zcol", [P, 1], f32r, kind="ExternalInput").ap()
    y_d = nc.dram_tensor("yT", [2, P, T], f32, kind="ExternalOutput").ap()

    with tile.TileContext(nc) as tc:
        _build(tc, (x_d, wq_d, wk_d, wv_d, bq_d, bk_d, wf_d, be_d, zc_d, y_d))
    nc.compile()
    _MODULE = nc
    return nc


def _marshal(x, wq, bq, wk, bk, wv, bv, w_fc, b_fc):
    """Host-side input prep (weights only -- all tiny)."""

    def blockdiag(w):
        # w: [E, E//H, KW] grouped conv weight ->
        # out[h, in_local, kk, out_local] block-diagonal per half.
        out = np.zeros((2, P, KW, P), np.float32)
        gs = E // H  # 32
        for h in range(2):
            for g in range(4):
                grp = 4 * h + g
                blk = w[gs * grp : gs * (grp + 1), :, :]  # [out c', in i, kk]
                for kk in range(KW):
                    out[h, gs * g : gs * (g + 1), kk, gs * g : gs * (g + 1)] = blk[
                        :, :, kk
                    ].T
        return out

    wqb = blockdiag(wq)
    wkb = blockdiag(wk)
    wvb = blockdiag(wv)
    bq2 = np.ascontiguousarray(bq.reshape(2, P).T)
    bk2 = np.ascontiguousarray(bk.reshape(2, P).T)
    wfcT = np.ascontiguousarray(w_fc.T.reshape(2, P, E))
    beff = np.ascontiguousarray(
        np.broadcast_to((w_fc @ bv + b_fc).reshape(1, E), (P, E))
    )
    return {
        "wqb": np.ascontiguousarray(wqb),
        "wkb": np.ascontiguousarray(wkb),
        "wvb": np.ascontiguousarray(wvb),
        "bq2": bq2,
        "bk2": bk2,
        "wfcT": wfcT,
        "beff": beff,
        "zcol": np.zeros((P, 1), np.float32),
    }


def kernel(x, wq, bq, wk, bk, wv, bv, w_fc, b_fc, num_heads):
    x = np.asarray(x, np.float32)
    consts = _marshal(
        x,
        np.asarray(wq, np.float32),
        np.asarray(bq, np.float32),
        np.asarray(wk, np.float32),
        np.asarray(bk, np.float32),
        np.asarray(wv, np.float32),
        np.asarray(bv, np.float32),
        np.asarray(w_fc, np.float32),
        np.asarray(b_fc, np.float32),
    )
    nc = build_module()
    in_maps = [{"x": np.ascontiguousarray(x[b]), **consts} for b in range(B)]
    res = run_bass_kernel_spmd(nc, in_maps, core_ids=list(range(NCORES)), trace=TRACE)
    LAST["exec_time_ns"] = res.exec_time_ns
    LAST["mean_exec_time_ns"] = res.mean_exec_time_ns
    LAST["results"] = res
    out = np.stack(
        [
            np.ascontiguousarray(res.results[b]["yT"].reshape(E, T).T)
            for b in range(B)
        ],
        axis=0,
    )
    return out


# revision 22
# speedup vs baseline: 1.1601x; 1.0225x over previous
"""ConvAttention kernel for 8x TRN2 NeuronCores.

Model (per batch item b):
    q/k/v = grouped_conv1d(x_b, w, b, groups=8)        # [E, T] -> [E, T]
    S     = (q^T k) / sqrt(E)                          # [T, T]
    P     = softmax(S, axis=-1)
    y     = (P @ v^T) @ w_fc^T + b_fc                  # [T, E]

Sharding: pure data-parallel over batch B=8 -> 8 cores, weights replicated.

Per-core algorithm (v2: fp8 DoubleRow on the T^2 matmuls):
  * conv projections as block-diagonal [128,128] fp32r matmuls per tap; q/k are
    written as fp8e4 in a DoubleRow-packed layout [128, 2, T] (the two
    128-channel halves on the middle axis).  q/k stay unit-variance (the
    1/sqrt(E) score scale is folded into the exp activation's `scale`).
  * vw' = v^T @ w_fc^T + beff (fc folded into v, conv bias bv + b_fc folded
    into beff which is added to every vw row: u' = u + r*beff, so
    u'/r = u/r + beff).  vw' stored fp8e4 [128, NST, 272].
  * S^T tiles per si-pair with ONE DoubleRow matmul per si (contraction 256 =
    both halves at once, 2 MACs/cell/cycle).
  * exp(S_raw/16 - 2) on the scalar engine straight to fp8e4 (the -2 shift
    guards fp8 overflow and cancels in the normalization).
  * attn@v runs TRANSPOSED with vw' as the DoubleRow stationary and the exp'd
    P^T pair tile as the moving operand: u'^T[e,t] accumulates over all 32
    si into 2 PSUM banks; a third M=1 DoubleRow matmul with a ones stationary
    accumulates the row sums r[t].
  * normalize: rinv = 1/r on DVE, partition-broadcast on GpSimd, y^T = u'^T *
    rinv on DVE, DMA'd out as y^T [2,128,T]; the host un-transposes.
"""

import contextlib

import numpy as np

import concourse.bacc as bacc
import concourse.mybir as mybir
import concourse.tile as tile
from concourse.bass_utils import run_bass_kernel_spmd

dt = mybir.dt
AF = mybir.ActivationFunctionType
DR = mybir.MatmulPerfMode.DoubleRow

B, E, T, H, KW = 8, 256, 4096, 8, 3
NCORES = 8
P = 128                  # partitions / half of E
TCH = 512                # t-chunk width
NCH = T // TCH           # 8 chunks
NST = T // P             # 32 s-tiles
NPAIR = NST // 2         # 16 si-pairs
VWW = 272                # vw row stride (>=E, multiple of 16 for DR ldweights)
SCALE = float(1.0 / np.sqrt(E))   # folded into the exp activation
SHIFT = -2.0             # exp shift; cancels in normalization, avoids fp8 ovf

TRACE = False
LAST = {}

_MODULE = None


def _build(tc, io):
    nc = tc.nc
    f32 = dt.float32
    f32r = dt.float32r
    f8 = dt.float8e4
    f16 = dt.float16
    x_d, wq_d, wk_d, wv_d, bq_d, bk_d, wf_d, be_d, zc_d, y_d = io
    wd_map = {0: wq_d, 1: wk_d, 2: wv_d}

    with contextlib.ExitStack() as ctx:
        const_p = ctx.enter_context(tc.tile_pool(name="const", bufs=1))
        x_p = ctx.enter_context(tc.tile_pool(name="xp", bufs=3))
        big_p = ctx.enter_context(tc.tile_pool(name="big", bufs=1))
        ch_p = ctx.enter_context(tc.tile_pool(name="ch", bufs=3))
        pt_p = ctx.enter_context(tc.tile_pool(name="ptp", bufs=4))
        out_p = ctx.enter_context(tc.tile_pool(name="outp", bufs=2))

        # Startup DMAs spread over 4 engine queues in need-order: the first
        # conv group (k, h=0, chunks 0+1) needs w(1,0) + x(j0,h0) + x(j1,h0).
        x0_tiles = {}
        xt = x_p.tile([P, TCH + 2], f32r, tag="x00", name="x0_0")
        nc.sync.dma_start(out=xt[:, 1 : TCH + 2], in_=x_d[0:P, 0 : TCH + 1])
        nc.sync.dma_start(out=xt[:, 0:1], in_=zc_d[:])
        x0_tiles[0] = xt
        xt = x_p.tile([P, TCH + 2], f32r, tag="x01", name="x0_1")
        nc.scalar.dma_start(out=xt[:, 1 : TCH + 2], in_=x_d[P : 2 * P, 0 : TCH + 1])
        nc.scalar.dma_start(out=xt[:, 0:1], in_=zc_d[:])
        x0_tiles[1] = xt

        # ---------------- constants ----------------
        bq_sb = const_p.tile([P, 2], f32, tag="bq", name="bq_sb")
        nc.gpsimd.dma_start(out=bq_sb[:], in_=bq_d[:])
        bk_sb = const_p.tile([P, 2], f32, tag="bk", name="bk_sb")
        nc.gpsimd.dma_start(out=bk_sb[:], in_=bk_d[:])
        w_sb = {}
        w_eng = {(1, 0): nc.sync, (1, 1): nc.scalar,
                 (0, 0): nc.gpsimd, (0, 1): nc.gpsimd,
                 (2, 0): nc.gpsimd, (2, 1): nc.gpsimd}
        for h in range(2):
            for pi in (1, 0, 2):
                wt = const_p.tile([P, KW, P], f32r, tag=f"w{pi}{h}", name=f"w{pi}{h}")
                w_eng[pi, h].dma_start(out=wt[:], in_=wd_map[pi][h])
                w_sb[pi, h] = wt
        wf_sb = []
        for h in range(2):
            wft = const_p.tile([P, E], f32r, tag=f"wf{h}", name=f"wf{h}")
            nc.gpsimd.dma_start(out=wft[:], in_=wf_d[h])
            wf_sb.append(wft)
        be_sb = const_p.tile([P, E], f32, tag="be", name="be_sb")
        nc.gpsimd.dma_start(out=be_sb[:], in_=be_d[:])

        # fp8 ones stationary for the row-sum matmul: [P, 2, 16], use [:, :, 0:1]
        ones_f = const_p.tile([P, 2], f32, tag="onef", name="ones_f")
        nc.vector.memset(ones_f[:], 1.0)
        # per-partition exp shift (bias operand for the activation)
        shift_sb = const_p.tile([P, 1], f32, tag="shift", name="shift_sb")
        nc.vector.memset(shift_sb[:], SHIFT)
        ones_pk = const_p.tile([P, 2, 16], f8, tag="onep", name="ones_pk")
        for i in range(2):
            nc.vector.tensor_copy(ones_pk[:, i, 0:1], ones_f[:, i : i + 1])

        # ---------------- resident tensors ----------------
        # q/k in fp16: fp8 scores dominate the error budget; fp16 streams at
        # the same 1 col/cycle as bf16 with 4 more mantissa bits.
        q_pk = big_p.tile([P, 2, T], f16, tag="qpk", name="q_pk")
        k_pk = big_p.tile([P, 2, T], f16, tag="kpk", name="k_pk")
        vw_sb = big_p.tile([P, NST, VWW], f8, tag="vw", name="vw_sb")

        def load_x_chunk(tag, h, j):
            xt = x_p.tile([P, TCH + 2], f32r, tag=f"{tag}{h}", name=f"{tag}{h}")
            rows = slice(h * P, (h + 1) * P)
            c0 = j * TCH - 1
            eng = nc.sync if h == 0 else nc.scalar
            if j == 0:
                eng.dma_start(out=xt[:, 0:1], in_=zc_d[:])
                eng.dma_start(out=xt[:, 1 : TCH + 2], in_=x_d[rows, 0 : TCH + 1])
            elif j == NCH - 1:
                eng.dma_start(out=xt[:, TCH + 1 : TCH + 2], in_=zc_d[:])
                eng.dma_start(out=xt[:, 0 : TCH + 1], in_=x_d[rows, c0:T])
            else:
                eng.dma_start(out=xt[:], in_=x_d[rows, c0 : c0 + TCH + 2])
            return xt

        # ---------------- phase 1: q, k, v -> vw' ----------------
        # chunk-paired: each conv weight tap is loaded once per two chunks
        with tc.tile_pool(name="ps_cv", bufs=2, space="PSUM") as ps_cv:
            for jp in range(NCH // 2):
                xts = {}
                for h in range(2):
                    for dj in range(2):
                        j = 2 * jp + dj
                        if j == 0 and h in x0_tiles:
                            xts[h, dj] = x0_tiles.pop(h)
                        else:
                            xts[h, dj] = load_x_chunk(f"x{dj}", h, j)
                v_ch = {}
                for h in range(2):
                    for pi in (1, 0, 2):
                        ps = {
                            dj: ps_cv.tile(
                                [P, TCH], f32, tag=f"cv{dj}", name=f"ps_cv{dj}"
                            )
                            for dj in range(2)
                        }
                        for kk in range(KW):
                            for dj in range(2):
                                nc.tensor.matmul(
                                    ps[dj][:],
                                    w_sb[pi, h][:, kk, :],
                                    xts[h, dj][:, kk : kk + TCH],
                                    start=(kk == 0),
                                    stop=(kk == KW - 1),
                                )
                        for dj in range(2):
                            j = 2 * jp + dj
                            tsl_c = slice(j * TCH, (j + 1) * TCH)
                            # q/k bias+cast on the (phase-1-idle) scalar
                            # engine; phase 1 is otherwise DVE-bound
                            if pi == 1:
                                nc.scalar.activation(
                                    k_pk[:, h, tsl_c],
                                    ps[dj][:],
                                    AF.Identity,
                                    bias=bk_sb[:, h : h + 1],
                                )
                            elif pi == 0:
                                nc.scalar.activation(
                                    q_pk[:, h, tsl_c],
                                    ps[dj][:],
                                    AF.Identity,
                                    bias=bq_sb[:, h : h + 1],
                                )
                            else:
                                vt = ch_p.tile(
                                    [P, TCH], f32r, tag=f"vch{h}{dj}", name=f"vch{h}{dj}"
                                )
                                nc.vector.tensor_copy(vt[:], ps[dj][:])
                                v_ch[h, dj] = vt
                for dj in range(2):
                    j = 2 * jp + dj
                    for ti in range(TCH // P):
                        si = j * (TCH // P) + ti
                        ps_vw = ps_cv.tile([P, E], f32, tag="vwp", name="ps_vw")
                        tsl = slice(ti * P, (ti + 1) * P)
                        nc.tensor.matmul(
                            ps_vw[:],
                            v_ch[0, dj][:, tsl],
                            wf_sb[0][:],
                            start=True,
                            stop=False,
                        )
                        nc.tensor.matmul(
                            ps_vw[:],
                            v_ch[1, dj][:, tsl],
                            wf_sb[1][:],
                            start=False,
                            stop=True,
                        )
                        # vw' = v@wfc^T + beff row-fold, cast to fp8
                        nc.vector.tensor_tensor(
                            vw_sb[:, si, 0:E], ps_vw[:], be_sb[:], mybir.AluOpType.add
                        )

        # ---------------- phase 2: attention ----------------
        # PSUM: st 2x1 + u 2x2 + r 2x1 = 8 banks.  ups/r double-buffered so
        # the normalize chain of chunk j overlaps chunk j+1's matmuls.
        with (
            tc.tile_pool(name="ps_st", bufs=2, space="PSUM") as ps_st,
            tc.tile_pool(name="ps_u", bufs=2, space="PSUM") as ps_u,
        ):
            for j in range(NCH):
                tslj = slice(j * TCH, (j + 1) * TCH)

                ups = [
                    ps_u.tile([P, TCH], f32, tag=f"u{h}", name=f"ups{h}")
                    for h in range(2)
                ]
                rps = ps_u.tile([P, TCH], f32, tag="r", name="rps")

                def st_d(p, d, pt):
                    """S^T fp16 matmuls + exp for si = 2p+d into pt plane d."""
                    ps = ps_st.tile([P, TCH], f32, tag="st", name="ps_stt")
                    ssl = slice((2 * p + d) * P, (2 * p + d + 1) * P)
                    for h in range(2):
                        nc.tensor.matmul(
                            ps[:],
                            k_pk[:, h, ssl],
                            q_pk[:, h, tslj],
                            start=(h == 0),
                            stop=(h == 1),
                        )
                    nc.scalar.activation(
                        pt[:, d, :], ps[:], AF.Exp, bias=shift_sb[:], scale=SCALE
                    )

                def u_h(p, pt, h):
                    nc.tensor.matmul(
                        ups[h][:],
                        vw_sb[:, 2 * p : 2 * p + 2, h * P : (h + 1) * P],
                        pt[:],
                        start=(p == 0),
                        stop=(p == NPAIR - 1),
                        perf_mode=DR,
                    )

                def u_r(p, pt):
                    nc.tensor.matmul(
                        rps[0:1, :],
                        ones_pk[:, :, 0:1],
                        pt[:],
                        start=(p == 0),
                        stop=(p == NPAIR - 1),
                        perf_mode=DR,
                    )

                # depth-2 software pipeline: u/r matmuls of pair p-2 are
                # interleaved between pair p's score matmuls, so they never
                # wait on the exp, and their double-width LDWEIGHTS hide
                # under the fp16 streams
                pts = {}
                for p in range(NPAIR):
                    pt = pt_p.tile([P, 2, TCH], f8, tag="pt", name="pt")
                    pts[p] = pt
                    st_d(p, 0, pt)
                    if p >= 2:
                        u_h(p - 2, pts[p - 2], 0)
                    st_d(p, 1, pt)
                    if p >= 2:
                        u_h(p - 2, pts[p - 2], 1)
                        u_r(p - 2, pts.pop(p - 2))
                for p in (NPAIR - 2, NPAIR - 1):
                    u_h(p, pts[p], 0)
                    u_h(p, pts[p], 1)
                    u_r(p, pts[p])

                # normalize: r -> SBUF -> broadcast (gpsimd) -> 1/r on all 128
                # DVE lanes (approx_fast: ~4e-6 rel, 5x faster than exact)
                rcp = out_p.tile([1, TCH], f32, tag="rcp", name="rcp")
                nc.vector.tensor_copy(rcp[:], rps[0:1, :])
                rbr = out_p.tile([P, TCH], f32, tag="rbr", name="rbr")
                nc.gpsimd.partition_broadcast(rbr[:], rcp[:])
                rb = out_p.tile([P, TCH], f32, tag="rb", name="rb")
                nc.vector.reciprocal_approx_fast(rb[:], rbr[:])
                for h in range(2):
                    yt = out_p.tile([P, TCH], f32, tag=f"yt{h}", name=f"yt{h}")
                    nc.vector.tensor_tensor(
                        yt[:], ups[h][:], rb[:], mybir.AluOpType.mult
                    )
                    nc.sync.dma_start(out=y_d[h][:, tslj], in_=yt[:])


def build_module():
    """Build + compile the Bass module (cached)."""
    global _MODULE
    if _MODULE is not None:
        return _MODULE
    nc = bacc.Bacc(
        "TRN2",
        target_bir_lowering=False,
        debug=False,
        enable_asserts=False,
        num_devices=NCORES,
    )
    f32 = dt.float32
    f32r = dt.float32r
    x_d = nc.dram_tensor("x", [E, T], f32r, kind="ExternalInput").ap()
    wq_d = nc.dram_tensor("wqb", [2, P, KW, P], f32r, kind="ExternalInput").ap()
    wk_d = nc.dram_tensor("wkb", [2, P, KW, P], f32r, kind="ExternalInput").ap()
    wv_d = nc.dram_tensor("wvb", [2, P, KW, P], f32r, kind="ExternalInput").ap()
    bq_d = nc.dram_tensor("bq2", [P, 2], f32, kind="ExternalInput").ap()
    bk_d = nc.dram_tensor("bk2", [P, 2], f32, kind="ExternalInput").ap()
    wf_d = nc.dram_tensor("wfcT", [2, P, E], f32r, kind="ExternalInput").ap()
    be_d = nc.dram_tensor("beff", [P, E], f32, kind="ExternalInput").ap()
    zc_d = nc.dram_tensor("zcol", [P, 1], f32r, kind="ExternalInput").ap()
    y_d = nc.dram_tensor("yT", [2, P, T], f32, kind="ExternalOutput").ap()

    with tile.TileContext(nc) as tc:
        _build(tc, (x_d, wq_d, wk_d, wv_d, bq_d, bk_d, wf_d, be_d, zc_d, y_d))
    nc.compile()
    _MODULE = nc
    return nc


def _marshal(x, wq, bq, wk, bk, wv, bv, w_fc, b_fc):
    """Host-side input prep (weights only -- all tiny)."""

    def blockdiag(w):
        # w: [E, E//H, KW] grouped conv weight ->
        # out[h, in_local, kk, out_local] block-diagonal per half.
        out = np.zeros((2, P, KW, P), np.float32)
        gs = E // H  # 32
        for h in range(2):
            for g in range(4):
                grp = 4 * h + g
                blk = w[gs * grp : gs * (grp + 1), :, :]  # [out c', in i, kk]
                for kk in range(KW):
                    out[h, gs * g : gs * (g + 1), kk, gs * g : gs * (g + 1)] = blk[
                        :, :, kk
                    ].T
        return out

    wqb = blockdiag(wq)
    wkb = blockdiag(wk)
    wvb = blockdiag(wv)
    bq2 = np.ascontiguousarray(bq.reshape(2, P).T)
    bk2 = np.ascontiguousarray(bk.reshape(2, P).T)
    wfcT = np.ascontiguousarray(w_fc.T.reshape(2, P, E))
    beff = np.ascontiguousarray(
        np.broadcast_to((w_fc @ bv + b_fc).reshape(1, E), (P, E))
    )
    return {
        "wqb": np.ascontiguousarray(wqb),
        "wkb": np.ascontiguousarray(wkb),
        "wvb": np.ascontiguousarray(wvb),
        "bq2": bq2,
        "bk2": bk2,
        "wfcT": wfcT,
        "beff": beff,
        "zcol": np.zeros((P, 1), np.float32),
    }


def kernel(x, wq, bq, wk, bk, wv, bv, w_fc, b_fc, num_heads):
    x = np.asarray(x, np.float32)
    consts = _marshal(
        x,
        np.asarray(wq, np.float32),
        np.asarray(bq, np.float32),
        np.asarray(wk, np.float32),
        np.asarray(bk, np.float32),
        np.asarray(wv, np.float32),
        np.asarray(bv, np.float32),
        np.asarray(w_fc, np.float32),
        np.asarray(b_fc, np.float32),
    )
    nc = build_module()
    in_maps = [{"x": np.ascontiguousarray(x[b]), **consts} for b in range(B)]
    res = run_bass_kernel_spmd(nc, in_maps, core_ids=list(range(NCORES)), trace=TRACE)
    LAST["exec_time_ns"] = res.exec_time_ns
    LAST["mean_exec_time_ns"] = res.mean_exec_time_ns
    LAST["results"] = res
    out = np.stack(
        [
            np.ascontiguousarray(res.results[b]["yT"].reshape(E, T).T)
            for b in range(B)
        ],
        axis=0,
    )
    return out


# revision 23
# speedup vs baseline: 1.1876x; 1.0237x over previous
"""ConvAttention kernel for 8x TRN2 NeuronCores.

Model (per batch item b):
    q/k/v = grouped_conv1d(x_b, w, b, groups=8)        # [E, T] -> [E, T]
    S     = (q^T k) / sqrt(E)                          # [T, T]
    P     = softmax(S, axis=-1)
    y     = (P @ v^T) @ w_fc^T + b_fc                  # [T, E]

Sharding: pure data-parallel over batch B=8 -> 8 cores, weights replicated.

Per-core algorithm (v2: fp8 DoubleRow on the T^2 matmuls):
  * conv projections as block-diagonal [128,128] fp32r matmuls per tap; q/k are
    written as fp8e4 in a DoubleRow-packed layout [128, 2, T] (the two
    128-channel halves on the middle axis).  q/k stay unit-variance (the
    1/sqrt(E) score scale is folded into the exp activation's `scale`).
  * vw' = v^T @ w_fc^T + beff (fc folded into v, conv bias bv + b_fc folded
    into beff which is added to every vw row: u' = u + r*beff, so
    u'/r = u/r + beff).  vw' stored fp8e4 [128, NST, 272].
  * S^T tiles per si-pair with ONE DoubleRow matmul per si (contraction 256 =
    both halves at once, 2 MACs/cell/cycle).
  * exp(S_raw/16 - 2) on the scalar engine straight to fp8e4 (the -2 shift
    guards fp8 overflow and cancels in the normalization).
  * attn@v runs TRANSPOSED with vw' as the DoubleRow stationary and the exp'd
    P^T pair tile as the moving operand: u'^T[e,t] accumulates over all 32
    si into 2 PSUM banks; a third M=1 DoubleRow matmul with a ones stationary
    accumulates the row sums r[t].
  * normalize: rinv = 1/r on DVE, partition-broadcast on GpSimd, y^T = u'^T *
    rinv on DVE, DMA'd out as y^T [2,128,T]; the host un-transposes.
"""

import contextlib

import numpy as np

import concourse.bacc as bacc
import concourse.mybir as mybir
import concourse.tile as tile
from concourse.bass_utils import run_bass_kernel_spmd

dt = mybir.dt
AF = mybir.ActivationFunctionType
DR = mybir.MatmulPerfMode.DoubleRow

B, E, T, H, KW = 8, 256, 4096, 8, 3
NCORES = 8
P = 128                  # partitions / half of E
TCH = 512                # t-chunk width
NCH = T // TCH           # 8 chunks
NST = T // P             # 32 s-tiles
NPAIR = NST // 2         # 16 si-pairs
VWW = 272                # vw row stride (>=E, multiple of 16 for DR ldweights)
SCALE = float(1.0 / np.sqrt(E))   # folded into the exp activation
SHIFT = -2.0             # exp shift; cancels in normalization, avoids fp8 ovf

TRACE = False
LAST = {}

_MODULE = None


def _build(tc, io):
    nc = tc.nc
    f32 = dt.float32
    f32r = dt.float32r
    f8 = dt.float8e4
    f16 = dt.float16
    x_d, wq_d, wk_d, wv_d, bq_d, bk_d, wf_d, be_d, zc_d, y_d = io
    wd_map = {0: wq_d, 1: wk_d, 2: wv_d}

    with contextlib.ExitStack() as ctx:
        const_p = ctx.enter_context(tc.tile_pool(name="const", bufs=1))
        x_p = ctx.enter_context(tc.tile_pool(name="xp", bufs=3))
        big_p = ctx.enter_context(tc.tile_pool(name="big", bufs=1))
        ch_p = ctx.enter_context(tc.tile_pool(name="ch", bufs=3))
        pt_p = ctx.enter_context(tc.tile_pool(name="ptp", bufs=4))
        out_p = ctx.enter_context(tc.tile_pool(name="outp", bufs=2))

        # Startup DMAs spread over 4 engine queues in need-order: the first
        # conv group (k, h=0, chunks 0+1) needs w(1,0) + x(j0,h0) + x(j1,h0).
        x0_tiles = {}
        xt = x_p.tile([P, TCH + 2], f32r, tag="x00", name="x0_0")
        nc.sync.dma_start(out=xt[:, 1 : TCH + 2], in_=x_d[0:P, 0 : TCH + 1])
        nc.sync.dma_start(out=xt[:, 0:1], in_=zc_d[:])
        x0_tiles[0] = xt
        xt = x_p.tile([P, TCH + 2], f32r, tag="x01", name="x0_1")
        nc.scalar.dma_start(out=xt[:, 1 : TCH + 2], in_=x_d[P : 2 * P, 0 : TCH + 1])
        nc.scalar.dma_start(out=xt[:, 0:1], in_=zc_d[:])
        x0_tiles[1] = xt

        # ---------------- constants ----------------
        bq_sb = const_p.tile([P, 2], f32, tag="bq", name="bq_sb")
        nc.gpsimd.dma_start(out=bq_sb[:], in_=bq_d[:])
        bk_sb = const_p.tile([P, 2], f32, tag="bk", name="bk_sb")
        nc.gpsimd.dma_start(out=bk_sb[:], in_=bk_d[:])
        w_sb = {}
        w_eng = {(1, 0): nc.sync, (1, 1): nc.scalar,
                 (0, 0): nc.gpsimd, (0, 1): nc.gpsimd,
                 (2, 0): nc.gpsimd, (2, 1): nc.gpsimd}
        for h in range(2):
            for pi in (1, 0, 2):
                wt = const_p.tile([P, KW, P], f32r, tag=f"w{pi}{h}", name=f"w{pi}{h}")
                w_eng[pi, h].dma_start(out=wt[:], in_=wd_map[pi][h])
                w_sb[pi, h] = wt
        wf_sb = []
        for h in range(2):
            wft = const_p.tile([P, E], f32r, tag=f"wf{h}", name=f"wf{h}")
            nc.gpsimd.dma_start(out=wft[:], in_=wf_d[h])
            wf_sb.append(wft)
        be_sb = const_p.tile([P, E], f32, tag="be", name="be_sb")
        nc.gpsimd.dma_start(out=be_sb[:], in_=be_d[:])

        # fp8 ones stationary for the row-sum matmul: [P, 2, 16], use [:, :, 0:1]
        ones_f = const_p.tile([P, 2], f32, tag="onef", name="ones_f")
        nc.vector.memset(ones_f[:], 1.0)
        # per-partition exp shift (bias operand for the activation)
        shift_sb = const_p.tile([P, 1], f32, tag="shift", name="shift_sb")
        nc.vector.memset(shift_sb[:], SHIFT)
        ones_pk = const_p.tile([P, 2, 16], f8, tag="onep", name="ones_pk")
        for i in range(2):
            nc.vector.tensor_copy(ones_pk[:, i, 0:1], ones_f[:, i : i + 1])

        # ---------------- resident tensors ----------------
        # q/k in fp16: fp8 scores dominate the error budget; fp16 streams at
        # the same 1 col/cycle as bf16 with 4 more mantissa bits.
        q_pk = big_p.tile([P, 2, T], f16, tag="qpk", name="q_pk")
        k_pk = big_p.tile([P, 2, T], f16, tag="kpk", name="k_pk")
        vw_sb = big_p.tile([P, NST, VWW], f8, tag="vw", name="vw_sb")

        def load_x_chunk(tag, h, j):
            xt = x_p.tile([P, TCH + 2], f32r, tag=f"{tag}{h}", name=f"{tag}{h}")
            rows = slice(h * P, (h + 1) * P)
            c0 = j * TCH - 1
            eng = nc.sync if h == 0 else nc.scalar
            if j == 0:
                eng.dma_start(out=xt[:, 0:1], in_=zc_d[:])
                eng.dma_start(out=xt[:, 1 : TCH + 2], in_=x_d[rows, 0 : TCH + 1])
            elif j == NCH - 1:
                eng.dma_start(out=xt[:, TCH + 1 : TCH + 2], in_=zc_d[:])
                eng.dma_start(out=xt[:, 0 : TCH + 1], in_=x_d[rows, c0:T])
            else:
                eng.dma_start(out=xt[:], in_=x_d[rows, c0 : c0 + TCH + 2])
            return xt

        # ---------------- phase 1: q, k, v -> vw' ----------------
        # chunk-paired: each conv weight tap is loaded once per two chunks
        with tc.tile_pool(name="ps_cv", bufs=2, space="PSUM") as ps_cv:
            for jp in range(NCH // 2):
                xts = {}
                for h in range(2):
                    for dj in range(2):
                        j = 2 * jp + dj
                        if j == 0 and h in x0_tiles:
                            xts[h, dj] = x0_tiles.pop(h)
                        else:
                            xts[h, dj] = load_x_chunk(f"x{dj}", h, j)
                v_ch = {}
                for h in range(2):
                    for pi in (1, 0, 2):
                        ps = {
                            dj: ps_cv.tile(
                                [P, TCH], f32, tag=f"cv{dj}", name=f"ps_cv{dj}"
                            )
                            for dj in range(2)
                        }
                        for kk in range(KW):
                            for dj in range(2):
                                nc.tensor.matmul(
                                    ps[dj][:],
                                    w_sb[pi, h][:, kk, :],
                                    xts[h, dj][:, kk : kk + TCH],
                                    start=(kk == 0),
                                    stop=(kk == KW - 1),
                                )
                        for dj in range(2):
                            j = 2 * jp + dj
                            tsl_c = slice(j * TCH, (j + 1) * TCH)
                            # q/k bias+cast on the (phase-1-idle) scalar
                            # engine; phase 1 is otherwise DVE-bound
                            if pi == 1:
                                nc.scalar.activation(
                                    k_pk[:, h, tsl_c],
                                    ps[dj][:],
                                    AF.Identity,
                                    bias=bk_sb[:, h : h + 1],
                                )
                            elif pi == 0:
                                nc.scalar.activation(
                                    q_pk[:, h, tsl_c],
                                    ps[dj][:],
                                    AF.Identity,
                                    bias=bq_sb[:, h : h + 1],
                                )
                            else:
                                vt = ch_p.tile(
                                    [P, TCH], f32r, tag=f"vch{h}{dj}", name=f"vch{h}{dj}"
                                )
                                nc.vector.tensor_copy(vt[:], ps[dj][:])
                                v_ch[h, dj] = vt
                for dj in range(2):
                    j = 2 * jp + dj
                    for ti in range(TCH // P):
                        si = j * (TCH // P) + ti
                        ps_vw = ps_cv.tile([P, E], f32, tag="vwp", name="ps_vw")
                        tsl = slice(ti * P, (ti + 1) * P)
                        nc.tensor.matmul(
                            ps_vw[:],
                            v_ch[0, dj][:, tsl],
                            wf_sb[0][:],
                            start=True,
                            stop=False,
                        )
                        nc.tensor.matmul(
                            ps_vw[:],
                            v_ch[1, dj][:, tsl],
                            wf_sb[1][:],
                            start=False,
                            stop=True,
                        )
                        # vw' = v@wfc^T + beff row-fold, cast to fp8
                        nc.vector.tensor_tensor(
                            vw_sb[:, si, 0:E], ps_vw[:], be_sb[:], mybir.AluOpType.add
                        )

        # ---------------- phase 2: attention ----------------
        # PSUM: st 2x1 + u 2x2 + r 2x1 = 8 banks.  ups/r double-buffered so
        # the normalize chain of chunk j overlaps chunk j+1's matmuls.
        with (
            tc.tile_pool(name="ps_st", bufs=2, space="PSUM") as ps_st,
            tc.tile_pool(name="ps_u", bufs=2, space="PSUM") as ps_u,
        ):
            for j in range(NCH):
                tslj = slice(j * TCH, (j + 1) * TCH)

                ups = [
                    ps_u.tile([P, TCH], f32, tag=f"u{h}", name=f"ups{h}")
                    for h in range(2)
                ]
                rps = ps_u.tile([P, TCH], f32, tag="r", name="rps", bufs=1)

                def st_d(p, d, pt):
                    """S^T fp16 matmuls + exp for si = 2p+d into pt plane d."""
                    ps = ps_st.tile([P, TCH], f32, tag="st", name="ps_stt", bufs=3)
                    ssl = slice((2 * p + d) * P, (2 * p + d + 1) * P)
                    for h in range(2):
                        nc.tensor.matmul(
                            ps[:],
                            k_pk[:, h, ssl],
                            q_pk[:, h, tslj],
                            start=(h == 0),
                            stop=(h == 1),
                        )
                    nc.scalar.activation(
                        pt[:, d, :], ps[:], AF.Exp, bias=shift_sb[:], scale=SCALE
                    )

                def u_h(p, pt, h):
                    nc.tensor.matmul(
                        ups[h][:],
                        vw_sb[:, 2 * p : 2 * p + 2, h * P : (h + 1) * P],
                        pt[:],
                        start=(p == 0),
                        stop=(p == NPAIR - 1),
                        perf_mode=DR,
                    )

                def u_r(p, pt):
                    nc.tensor.matmul(
                        rps[0:1, :],
                        ones_pk[:, :, 0:1],
                        pt[:],
                        start=(p == 0),
                        stop=(p == NPAIR - 1),
                        perf_mode=DR,
                    )

                # depth-2 software pipeline: u/r matmuls of pair p-2 are
                # interleaved between pair p's score matmuls, so they never
                # wait on the exp, and their double-width LDWEIGHTS hide
                # under the fp16 streams
                pts = {}
                for p in range(NPAIR):
                    pt = pt_p.tile([P, 2, TCH], f8, tag="pt", name="pt")
                    pts[p] = pt
                    st_d(p, 0, pt)
                    if p >= 2:
                        u_h(p - 2, pts[p - 2], 0)
                    st_d(p, 1, pt)
                    if p >= 2:
                        u_h(p - 2, pts[p - 2], 1)
                        u_r(p - 2, pts.pop(p - 2))
                for p in (NPAIR - 2, NPAIR - 1):
                    u_h(p, pts[p], 0)
                    u_h(p, pts[p], 1)
                    u_r(p, pts[p])

                # normalize: r -> SBUF -> broadcast (gpsimd) -> 1/r on all 128
                # DVE lanes (approx_fast: ~4e-6 rel, 5x faster than exact)
                rcp = out_p.tile([1, TCH], f32, tag="rcp", name="rcp")
                nc.vector.tensor_copy(rcp[:], rps[0:1, :])
                rbr = out_p.tile([P, TCH], f32, tag="rbr", name="rbr")
                nc.gpsimd.partition_broadcast(rbr[:], rcp[:])
                rb = out_p.tile([P, TCH], f32, tag="rb", name="rb")
                nc.vector.reciprocal_approx_fast(rb[:], rbr[:])
                for h in range(2):
                    yt = out_p.tile([P, TCH], f32, tag=f"yt{h}", name=f"yt{h}")
                    nc.vector.tensor_tensor(
                        yt[:], ups[h][:], rb[:], mybir.AluOpType.mult
                    )
                    nc.sync.dma_start(out=y_d[h][:, tslj], in_=yt[:])


def build_module():
    """Build + compile the Bass module (cached)."""
    global _MODULE
    if _MODULE is not None:
        return _MODULE
    nc = bacc.Bacc(
        "TRN2",
        target_bir_lowering=False,
        debug=False,
        enable_asserts=False,
        num_devices=NCORES,
    )
    f32 = dt.float32
    f32r = dt.float32r
    x_d = nc.dram_tensor("x", [E, T], f32r, kind="ExternalInput").ap()
    wq_d = nc.dram_tensor("wqb", [2, P, KW, P], f32r, kind="ExternalInput").ap()
    wk_d = nc.dram_tensor("wkb", [2, P, KW, P], f32r, kind="ExternalInput").ap()
    wv_d = nc.dram_tensor("wvb", [2, P, KW, P], f32r, kind="ExternalInput").ap()
    bq_d = nc.dram_tensor("bq2", [P, 2], f32, kind="ExternalInput").ap()
    bk_d = nc.dram_tensor("bk2", [P, 2], f32, kind="ExternalInput").ap()
    wf_d = nc.dram_tensor("wfcT", [2, P, E], f32r, kind="ExternalInput").ap()
    be_d = nc.dram_tensor("beff", [P, E], f32, kind="ExternalInput").ap()
    zc_d = nc.dram_tensor("zcol", [P, 1], f32r, kind="ExternalInput").ap()
    y_d = nc.dram_tensor("yT", [2, P, T], f32, kind="ExternalOutput").ap()

    with tile.TileContext(nc) as tc:
        _build(tc, (x_d, wq_d, wk_d, wv_d, bq_d, bk_d, wf_d, be_d, zc_d, y_d))
    nc.compile()
    _MODULE = nc
    return nc


def _marshal(x, wq, bq, wk, bk, wv, bv, w_fc, b_fc):
    """Host-side input prep (weights only -- all tiny)."""

    def blockdiag(w):
        # w: [E, E//H, KW] grouped conv weight ->
        # out[h, in_local, kk, out_local] block-diagonal per half.
        out = np.zeros((2, P, KW, P), np.float32)
        gs = E // H  # 32
        for h in range(2):
            for g in range(4):
                grp = 4 * h + g
                blk = w[gs * grp : gs * (grp + 1), :, :]  # [out c', in i, kk]
                for kk in range(KW):
                    out[h, gs * g : gs * (g + 1), kk, gs * g : gs * (g + 1)] = blk[
                        :, :, kk
                    ].T
        return out

    wqb = blockdiag(wq)
    wkb = blockdiag(wk)
    wvb = blockdiag(wv)
    bq2 = np.ascontiguousarray(bq.reshape(2, P).T)
    bk2 = np.ascontiguousarray(bk.reshape(2, P).T)
    wfcT = np.ascontiguousarray(w_fc.T.reshape(2, P, E))
    beff = np.ascontiguousarray(
        np.broadcast_to((w_fc @ bv + b_fc).reshape(1, E), (P, E))
    )
    return {
        "wqb": np.ascontiguousarray(wqb),
        "wkb": np.ascontiguousarray(wkb),
        "wvb": np.ascontiguousarray(wvb),
        "bq2": bq2,
        "bk2": bk2,
        "wfcT": wfcT,
        "beff": beff,
        "zcol": np.zeros((P, 1), np.float32),
    }


def kernel(x, wq, bq, wk, bk, wv, bv, w_fc, b_fc, num_heads):
    x = np.asarray(x, np.float32)
    consts = _marshal(
        x,
        np.asarray(wq, np.float32),
        np.asarray(bq, np.float32),
        np.asarray(wk, np.float32),
        np.asarray(bk, np.float32),
        np.asarray(wv, np.float32),
        np.asarray(bv, np.float32),
        np.asarray(w_fc, np.float32),
        np.asarray(b_fc, np.float32),
    )
    nc = build_module()
    in_maps = [{"x": np.ascontiguousarray(x[b]), **consts} for b in range(B)]
    res = run_bass_kernel_spmd(nc, in_maps, core_ids=list(range(NCORES)), trace=TRACE)
    LAST["exec_time_ns"] = res.exec_time_ns
    LAST["mean_exec_time_ns"] = res.mean_exec_time_ns
    LAST["results"] = res
    out = np.stack(
        [
            np.ascontiguousarray(res.results[b]["yT"].reshape(E, T).T)
            for b in range(B)
        ],
        axis=0,
    )
    return out


# revision 25
# speedup vs baseline: 1.1992x; 1.0098x over previous
"""ConvAttention kernel for 8x TRN2 NeuronCores.

Model (per batch item b):
    q/k/v = grouped_conv1d(x_b, w, b, groups=8)        # [E, T] -> [E, T]
    S     = (q^T k) / sqrt(E)                          # [T, T]
    P     = softmax(S, axis=-1)
    y     = (P @ v^T) @ w_fc^T + b_fc                  # [T, E]

Sharding: pure data-parallel over batch B=8 -> 8 cores, weights replicated.

Per-core algorithm (v2: fp8 DoubleRow on the T^2 matmuls):
  * conv projections as block-diagonal [128,128] fp32r matmuls per tap; q/k are
    written as fp8e4 in a DoubleRow-packed layout [128, 2, T] (the two
    128-channel halves on the middle axis).  q/k stay unit-variance (the
    1/sqrt(E) score scale is folded into the exp activation's `scale`).
  * vw' = v^T @ w_fc^T + beff (fc folded into v, conv bias bv + b_fc folded
    into beff which is added to every vw row: u' = u + r*beff, so
    u'/r = u/r + beff).  vw' stored fp8e4 [128, NST, 272].
  * S^T tiles per si-pair with ONE DoubleRow matmul per si (contraction 256 =
    both halves at once, 2 MACs/cell/cycle).
  * exp(S_raw/16 - 2) on the scalar engine straight to fp8e4 (the -2 shift
    guards fp8 overflow and cancels in the normalization).
  * attn@v runs TRANSPOSED with vw' as the DoubleRow stationary and the exp'd
    P^T pair tile as the moving operand: u'^T[e,t] accumulates over all 32
    si into 2 PSUM banks; a third M=1 DoubleRow matmul with a ones stationary
    accumulates the row sums r[t].
  * normalize: rinv = 1/r on DVE, partition-broadcast on GpSimd, y^T = u'^T *
    rinv on DVE, DMA'd out as y^T [2,128,T]; the host un-transposes.
"""

import contextlib

import numpy as np

import concourse.bacc as bacc
import concourse.mybir as mybir
import concourse.tile as tile
from concourse.bass_utils import run_bass_kernel_spmd

dt = mybir.dt
AF = mybir.ActivationFunctionType
DR = mybir.MatmulPerfMode.DoubleRow

B, E, T, H, KW = 8, 256, 4096, 8, 3
NCORES = 8
P = 128                  # partitions / half of E
TCH = 512                # t-chunk width
NCH = T // TCH           # 8 chunks
NST = T // P             # 32 s-tiles
NPAIR = NST // 2         # 16 si-pairs
VWW = 272                # vw row stride (>=E, multiple of 16 for DR ldweights)
SCALE = float(1.0 / np.sqrt(E))   # folded into the exp activation
SHIFT = -2.0             # exp shift; cancels in normalization, avoids fp8 ovf

TRACE = False
LAST = {}

_MODULE = None


def _build(tc, io):
    nc = tc.nc
    f32 = dt.float32
    f32r = dt.float32r
    f8 = dt.float8e4
    f16 = dt.float16
    x_d, wq_d, wk_d, wv_d, bq_d, bk_d, wf_d, be_d, zc_d, y_d = io
    wd_map = {0: wq_d, 1: wk_d, 2: wv_d}

    with contextlib.ExitStack() as ctx:
        const_p = ctx.enter_context(tc.tile_pool(name="const", bufs=1))
        x_p = ctx.enter_context(tc.tile_pool(name="xp", bufs=3))
        big_p = ctx.enter_context(tc.tile_pool(name="big", bufs=1))
        ch_p = ctx.enter_context(tc.tile_pool(name="ch", bufs=3))
        pt_p = ctx.enter_context(tc.tile_pool(name="ptp", bufs=4))
        out_p = ctx.enter_context(tc.tile_pool(name="outp", bufs=2))

        # Startup DMAs spread over 3 DMA-capable queues in need-order: the
        # first conv group (k, h=0, chunks 0+1) needs w(1,0) + x(j0,h0) +
        # x(j1,h0); the tap loop interleaves both chunks, so x(j1,h0) must
        # not queue behind anything big.
        x0_tiles = {}
        xt = x_p.tile([P, TCH + 2], f32r, tag="x00", name="x0_0")
        nc.sync.dma_start(out=xt[:, 1 : TCH + 2], in_=x_d[0:P, 0 : TCH + 1])
        nc.sync.dma_start(out=xt[:, 0:1], in_=zc_d[:])
        x0_tiles[0] = xt
        x1_tiles = {}
        xt = x_p.tile([P, TCH + 2], f32r, tag="x10", name="x1_0")
        nc.scalar.dma_start(out=xt[:], in_=x_d[0:P, TCH - 1 : 2 * TCH + 1])
        x1_tiles[0] = xt
        xt = x_p.tile([P, TCH + 2], f32r, tag="x01", name="x0_1")
        nc.scalar.dma_start(out=xt[:, 1 : TCH + 2], in_=x_d[P : 2 * P, 0 : TCH + 1])
        nc.scalar.dma_start(out=xt[:, 0:1], in_=zc_d[:])
        x0_tiles[1] = xt

        # ---------------- constants ----------------
        bq_sb = const_p.tile([P, 2], f32, tag="bq", name="bq_sb")
        nc.gpsimd.dma_start(out=bq_sb[:], in_=bq_d[:])
        bk_sb = const_p.tile([P, 2], f32, tag="bk", name="bk_sb")
        nc.gpsimd.dma_start(out=bk_sb[:], in_=bk_d[:])
        w_sb = {}
        w_eng = {(1, 0): nc.sync, (1, 1): nc.scalar,
                 (0, 0): nc.gpsimd, (0, 1): nc.gpsimd,
                 (2, 0): nc.gpsimd, (2, 1): nc.gpsimd}
        for h in range(2):
            for pi in (1, 0, 2):
                wt = const_p.tile([P, KW, P], f32r, tag=f"w{pi}{h}", name=f"w{pi}{h}")
                w_eng[pi, h].dma_start(out=wt[:], in_=wd_map[pi][h])
                w_sb[pi, h] = wt
        wf_sb = []
        for h in range(2):
            wft = const_p.tile([P, E], f32r, tag=f"wf{h}", name=f"wf{h}")
            nc.gpsimd.dma_start(out=wft[:], in_=wf_d[h])
            wf_sb.append(wft)
        be_sb = const_p.tile([P, E], f32, tag="be", name="be_sb")
        nc.gpsimd.dma_start(out=be_sb[:], in_=be_d[:])

        # fp8 ones stationary for the row-sum matmul: [P, 2, 16], use [:, :, 0:1]
        ones_f = const_p.tile([P, 2], f32, tag="onef", name="ones_f")
        nc.vector.memset(ones_f[:], 1.0)
        # per-partition exp shift (bias operand for the activation)
        shift_sb = const_p.tile([P, 1], f32, tag="shift", name="shift_sb")
        nc.vector.memset(shift_sb[:], SHIFT)
        ones_pk = const_p.tile([P, 2, 16], f8, tag="onep", name="ones_pk")
        for i in range(2):
            nc.vector.tensor_copy(ones_pk[:, i, 0:1], ones_f[:, i : i + 1])

        # ---------------- resident tensors ----------------
        # q/k in fp16: fp8 scores dominate the error budget; fp16 streams at
        # the same 1 col/cycle as bf16 with 4 more mantissa bits.
        q_pk = big_p.tile([P, 2, T], f16, tag="qpk", name="q_pk")
        k_pk = big_p.tile([P, 2, T], f16, tag="kpk", name="k_pk")
        vw_sb = big_p.tile([P, NST, VWW], f8, tag="vw", name="vw_sb")

        def load_x_chunk(tag, h, j):
            xt = x_p.tile([P, TCH + 2], f32r, tag=f"{tag}{h}", name=f"{tag}{h}")
            rows = slice(h * P, (h + 1) * P)
            c0 = j * TCH - 1
            eng = nc.sync if h == 0 else nc.scalar
            if j == 0:
                eng.dma_start(out=xt[:, 0:1], in_=zc_d[:])
                eng.dma_start(out=xt[:, 1 : TCH + 2], in_=x_d[rows, 0 : TCH + 1])
            elif j == NCH - 1:
                eng.dma_start(out=xt[:, TCH + 1 : TCH + 2], in_=zc_d[:])
                eng.dma_start(out=xt[:, 0 : TCH + 1], in_=x_d[rows, c0:T])
            else:
                eng.dma_start(out=xt[:], in_=x_d[rows, c0 : c0 + TCH + 2])
            return xt

        # ---------------- phase 1: q, k, v -> vw' ----------------
        # chunk-paired: each conv weight tap is loaded once per two chunks
        with tc.tile_pool(name="ps_cv", bufs=2, space="PSUM") as ps_cv:
            for jp in range(NCH // 2):
                xts = {}
                for h in range(2):
                    for dj in range(2):
                        j = 2 * jp + dj
                        if j == 0 and h in x0_tiles:
                            xts[h, dj] = x0_tiles.pop(h)
                        elif j == 1 and h in x1_tiles:
                            xts[h, dj] = x1_tiles.pop(h)
                        else:
                            xts[h, dj] = load_x_chunk(f"x{dj}", h, j)
                v_ch = {}
                for h in range(2):
                    for pi in (1, 0, 2):
                        ps = {
                            dj: ps_cv.tile(
                                [P, TCH], f32, tag=f"cv{dj}", name=f"ps_cv{dj}"
                            )
                            for dj in range(2)
                        }
                        for kk in range(KW):
                            for dj in range(2):
                                nc.tensor.matmul(
                                    ps[dj][:],
                                    w_sb[pi, h][:, kk, :],
                                    xts[h, dj][:, kk : kk + TCH],
                                    start=(kk == 0),
                                    stop=(kk == KW - 1),
                                )
                        for dj in range(2):
                            j = 2 * jp + dj
                            tsl_c = slice(j * TCH, (j + 1) * TCH)
                            # q/k bias+cast on the (phase-1-idle) scalar
                            # engine; phase 1 is otherwise DVE-bound
                            if pi == 1:
                                nc.scalar.activation(
                                    k_pk[:, h, tsl_c],
                                    ps[dj][:],
                                    AF.Identity,
                                    bias=bk_sb[:, h : h + 1],
                                )
                            elif pi == 0:
                                nc.scalar.activation(
                                    q_pk[:, h, tsl_c],
                                    ps[dj][:],
                                    AF.Identity,
                                    bias=bq_sb[:, h : h + 1],
                                )
                            else:
                                vt = ch_p.tile(
                                    [P, TCH], f32r, tag=f"vch{h}{dj}", name=f"vch{h}{dj}"
                                )
                                nc.vector.tensor_copy(vt[:], ps[dj][:])
                                v_ch[h, dj] = vt
                for dj in range(2):
                    j = 2 * jp + dj
                    for ti in range(TCH // P):
                        si = j * (TCH // P) + ti
                        ps_vw = ps_cv.tile([P, E], f32, tag="vwp", name="ps_vw")
                        tsl = slice(ti * P, (ti + 1) * P)
                        nc.tensor.matmul(
                            ps_vw[:],
                            v_ch[0, dj][:, tsl],
                            wf_sb[0][:],
                            start=True,
                            stop=False,
                        )
                        nc.tensor.matmul(
                            ps_vw[:],
                            v_ch[1, dj][:, tsl],
                            wf_sb[1][:],
                            start=False,
                            stop=True,
                        )
                        # vw' = v@wfc^T + beff row-fold, cast to fp8
                        nc.vector.tensor_tensor(
                            vw_sb[:, si, 0:E], ps_vw[:], be_sb[:], mybir.AluOpType.add
                        )

        # ---------------- phase 2: attention ----------------
        # PSUM: st 2x1 + u 2x2 + r 2x1 = 8 banks.  ups/r double-buffered so
        # the normalize chain of chunk j overlaps chunk j+1's matmuls.
        with (
            tc.tile_pool(name="ps_st", bufs=2, space="PSUM") as ps_st,
            tc.tile_pool(name="ps_u", bufs=2, space="PSUM") as ps_u,
        ):
            for j in range(NCH):
                tslj = slice(j * TCH, (j + 1) * TCH)

                ups = [
                    ps_u.tile([P, TCH], f32, tag=f"u{h}", name=f"ups{h}")
                    for h in range(2)
                ]
                rps = ps_u.tile([P, TCH], f32, tag="r", name="rps", bufs=1)

                def st_d(p, d, pt):
                    """S^T fp16 matmuls + exp for si = 2p+d into pt plane d."""
                    ps = ps_st.tile([P, TCH], f32, tag="st", name="ps_stt", bufs=3)
                    ssl = slice((2 * p + d) * P, (2 * p + d + 1) * P)
                    for h in range(2):
                        nc.tensor.matmul(
                            ps[:],
                            k_pk[:, h, ssl],
                            q_pk[:, h, tslj],
                            start=(h == 0),
                            stop=(h == 1),
                        )
                    nc.scalar.activation(
                        pt[:, d, :], ps[:], AF.Exp, bias=shift_sb[:], scale=SCALE
                    )

                def u_h(p, pt, h):
                    nc.tensor.matmul(
                        ups[h][:],
                        vw_sb[:, 2 * p : 2 * p + 2, h * P : (h + 1) * P],
                        pt[:],
                        start=(p == 0),
                        stop=(p == NPAIR - 1),
                        perf_mode=DR,
                    )

                def u_r(p, pt):
                    nc.tensor.matmul(
                        rps[0:1, :],
                        ones_pk[:, :, 0:1],
                        pt[:],
                        start=(p == 0),
                        stop=(p == NPAIR - 1),
                        perf_mode=DR,
                    )

                # depth-2 software pipeline: u/r matmuls of pair p-2 are
                # interleaved between pair p's score matmuls, so they never
                # wait on the exp, and their double-width LDWEIGHTS hide
                # under the fp16 streams
                pts = {}
                for p in range(NPAIR):
                    pt = pt_p.tile([P, 2, TCH], f8, tag="pt", name="pt")
                    pts[p] = pt
                    st_d(p, 0, pt)
                    if p >= 2:
                        u_h(p - 2, pts[p - 2], 0)
                    st_d(p, 1, pt)
                    if p >= 2:
                        u_h(p - 2, pts[p - 2], 1)
                        u_r(p - 2, pts.pop(p - 2))
                for p in (NPAIR - 2, NPAIR - 1):
                    u_h(p, pts[p], 0)
                    u_h(p, pts[p], 1)
                    u_r(p, pts[p])

                # normalize: r -> SBUF -> broadcast (gpsimd) -> 1/r on all 128
                # DVE lanes (approx_fast: ~4e-6 rel, 5x faster than exact)
                rcp = out_p.tile([1, TCH], f32, tag="rcp", name="rcp")
                nc.vector.tensor_copy(rcp[:], rps[0:1, :])
                rbr = out_p.tile([P, TCH], f32, tag="rbr", name="rbr")
                nc.gpsimd.partition_broadcast(rbr[:], rcp[:])
                rb = out_p.tile([P, TCH], f32, tag="rb", name="rb")
                nc.vector.reciprocal_approx_fast(rb[:], rbr[:])
                for h in range(2):
                    yt = out_p.tile([P, TCH], f32, tag=f"yt{h}", name=f"yt{h}")
                    nc.vector.tensor_tensor(
                        yt[:], ups[h][:], rb[:], mybir.AluOpType.mult
                    )
                    nc.sync.dma_start(out=y_d[h][:, tslj], in_=yt[:])


def build_module():
    """Build + compile the Bass module (cached)."""
    global _MODULE
    if _MODULE is not None:
        return _MODULE
    nc = bacc.Bacc(
        "TRN2",
        target_bir_lowering=False,
        debug=False,
        enable_asserts=False,
        num_devices=NCORES,
    )
    f32 = dt.float32
    f32r = dt.float32r
    x_d = nc.dram_tensor("x", [E, T], f32r, kind="ExternalInput").ap()
    wq_d = nc.dram_tensor("wqb", [2, P, KW, P], f32r, kind="ExternalInput").ap()
    wk_d = nc.dram_tensor("wkb", [2, P, KW, P], f32r, kind="ExternalInput").ap()
    wv_d = nc.dram_tensor("wvb", [2, P, KW, P], f32r, kind="ExternalInput").ap()
    bq_d = nc.dram_tensor("bq2", [P, 2], f32, kind="ExternalInput").ap()
    bk_d = nc.dram_tensor("bk2", [P, 2], f32, kind="ExternalInput").ap()
    wf_d = nc.dram_tensor("wfcT", [2, P, E], f32r, kind="ExternalInput").ap()
    be_d = nc.dram_tensor("beff", [P, E], f32, kind="ExternalInput").ap()
    zc_d = nc.dram_tensor("zcol", [P, 1], f32r, kind="ExternalInput").ap()
    y_d = nc.dram_tensor("yT", [2, P, T], f32, kind="ExternalOutput").ap()

    with tile.TileContext(nc) as tc:
        _build(tc, (x_d, wq_d, wk_d, wv_d, bq_d, bk_d, wf_d, be_d, zc_d, y_d))
    nc.compile()
    _MODULE = nc
    return nc


def _marshal(x, wq, bq, wk, bk, wv, bv, w_fc, b_fc):
    """Host-side input prep (weights only -- all tiny)."""

    def blockdiag(w):
        # w: [E, E//H, KW] grouped conv weight ->
        # out[h, in_local, kk, out_local] block-diagonal per half.
        out = np.zeros((2, P, KW, P), np.float32)
        gs = E // H  # 32
        for h in range(2):
            for g in range(4):
                grp = 4 * h + g
                blk = w[gs * grp : gs * (grp + 1), :, :]  # [out c', in i, kk]
                for kk in range(KW):
                    out[h, gs * g : gs * (g + 1), kk, gs * g : gs * (g + 1)] = blk[
                        :, :, kk
                    ].T
        return out

    wqb = blockdiag(wq)
    wkb = blockdiag(wk)
    wvb = blockdiag(wv)
    bq2 = np.ascontiguousarray(bq.reshape(2, P).T)
    bk2 = np.ascontiguousarray(bk.reshape(2, P).T)
    wfcT = np.ascontiguousarray(w_fc.T.reshape(2, P, E))
    beff = np.ascontiguousarray(
        np.broadcast_to((w_fc @ bv + b_fc).reshape(1, E), (P, E))
    )
    return {
        "wqb": np.ascontiguousarray(wqb),
        "wkb": np.ascontiguousarray(wkb),
        "wvb": np.ascontiguousarray(wvb),
        "bq2": bq2,
        "bk2": bk2,
        "wfcT": wfcT,
        "beff": beff,
        "zcol": np.zeros((P, 1), np.float32),
    }


def kernel(x, wq, bq, wk, bk, wv, bv, w_fc, b_fc, num_heads):
    x = np.asarray(x, np.float32)
    consts = _marshal(
        x,
        np.asarray(wq, np.float32),
        np.asarray(bq, np.float32),
        np.asarray(wk, np.float32),
        np.asarray(bk, np.float32),
        np.asarray(wv, np.float32),
        np.asarray(bv, np.float32),
        np.asarray(w_fc, np.float32),
        np.asarray(b_fc, np.float32),
    )
    nc = build_module()
    in_maps = [{"x": np.ascontiguousarray(x[b]), **consts} for b in range(B)]
    res = run_bass_kernel_spmd(nc, in_maps, core_ids=list(range(NCORES)), trace=TRACE)
    LAST["exec_time_ns"] = res.exec_time_ns
    LAST["mean_exec_time_ns"] = res.mean_exec_time_ns
    LAST["results"] = res
    out = np.stack(
        [
            np.ascontiguousarray(res.results[b]["yT"].reshape(E, T).T)
            for b in range(B)
        ],
        axis=0,
    )
    return out
